# revision 3
# baseline (speedup 1.0000x reference)
"""MQA attention (32 query heads, 1 KV head, ALiBi, causal) on 8 trn2 cores.

Sharding: tensor-parallel over query heads (4 heads/core). Wq rows and Wo
columns are sharded; x, Wk, Wv are replicated. Each core computes a partial
[T, E] output (its 4 heads pushed through its Wo column-shard); the host sums
the 8 partials.

v2 design vs baseline:
- bf16 operands everywhere on the PE (1 cyc/col at any width, no fp32r
  256-col padding); fp32 accumulation in PSUM; partial output written bf16.
- ALiBi-windowed attention: head h's weights decay as exp(-s_h * dist), so
  keys beyond dist tau/s_h contribute < e^-20 relative and are skipped at
  128-block granularity. Cores get head sets {24+c, 16+c, 8+c, c} with
  identical window profiles W = [16(full), 10, 3, 1] blocks, so the SPMD
  instruction stream is core-independent and load-balanced.
- Few large DMAs (HWDGE serializes at ~625ns/DMA): x loaded in big strided
  DMAs, weights in 4, output staged to [128, 2048] bf16 rows.
- The PE executes strictly in program order, so emission order IS the
  schedule: score->AV skew of 4 tiles, AV/outproj work deferred across
  phase boundaries as filler between other PE ops, so the PE never sits
  behind a parked instruction waiting on the ACT/Pool exp/mask chain.

Math per core c (slots j=0..3, heads hs=[24+c, 16+c, 8+c, c]):
  qT_j = (Wq_hj * D^-0.5) @ x^T                    [64, T]
  kT   = Wk @ x^T, v = (Wv @ x^T)^T                [64, T], [T, 64]
  ST[j, i] = kT[:,j].q  +  (-s_h * i)              via augmented row (ones in
                                                   kTa row 64, -s_h*i in qTa)
  g = exp(ST + s_h*j)      (per-partition ACT bias; per-column factors cancel
                            in softmax normalization; causal mask via
                            affine_select on diagonal blocks; key blocks
                            outside the ALiBi window skipped)
  OT_aug = [v | 1]^T @ g   -> rows 0:64 = unnormalized head out^T,
                              row 64    = softmax denominator
  headout^T = OT / denom   (partition_broadcast of 1/denom)
  partial = headout^T.T @ WoT_shard                [T, E] bf16
"""

from collections import deque

import numpy as np
import ml_dtypes

import concourse.bacc as bacc
import concourse.bass as bass
import concourse.mybir as mybir
import concourse.tile as tile
from concourse.masks import make_identity
from concourse.bass_utils import run_bass_kernel_spmd

T = 2048          # tokens
E = 2048          # embed dim
H = 32            # query heads
D = 64            # head dim
NCORES = 8
HL = H // NCORES  # 4 heads per core
ES = HL * D       # 256 = per-core E shard
TQ = 512          # query-phase tile
NTQ = T // TQ     # 4
NE = E // 128     # 16 contraction chunks
NT128 = T // 128  # 16

WS = [16, 1, 10, 3]   # per-slot ALiBi windows (key blocks beyond diagonal);
                      # pairs (0,1) and (2,3) balance long+short tile lists
SKEW = 4              # score -> AV deferral depth (tiles)

F32 = mybir.dt.float32
BF16 = mybir.dt.bfloat16
EXP = mybir.ActivationFunctionType.Exp
NPBF16 = ml_dtypes.bfloat16

_CACHE = {}


def _tk_ranges(q, W):
    """(tk, lo, hi) global-column score tiles for query block q, window W."""
    cs, ce = q * TQ, (q + 1) * TQ
    out = []
    for tk in range(max(0, 4 * q - W), 4 * q + 4):
        lo = max(cs, tk * 128)
        hi = min(ce, (tk + W + 1) * 128)
        out.append((tk, lo, hi))
    return out


def _build_nc():
    nc = bacc.Bacc("TRN2")
    xT = nc.dram_tensor("xT", [E, T], BF16, kind="ExternalInput")
    wqkvT = nc.dram_tensor("wqkvT", [E, ES + 2 * D], BF16, kind="ExternalInput")
    woT = nc.dram_tensor("woT", [ES, E], BF16, kind="ExternalInput")
    qrow = nc.dram_tensor("qrow", [HL, T], BF16, kind="ExternalInput")
    btbl = nc.dram_tensor("btbl", [128, HL * NT128], F32, kind="ExternalInput")
    part = nc.dram_tensor("part", [T, E], BF16, kind="ExternalOutput")

    from contextlib import ExitStack
    with tile.TileContext(nc) as tc, ExitStack() as ctx:
        _body(nc, tc, ctx, xT, wqkvT, woT, qrow, btbl, part)
    nc.finalize()
    return nc


class _K:
    """Kernel emission state: tile pools, resident tiles, and the deferral
    queues. The PE runs strictly in program order, so emission order is the
    schedule: attention tiles (whose g comes back through the ACT/Pool
    exp/mask chain) are interleaved beat-by-beat with "dense" PE work
    (projection chains, output projection) that has no cross-engine latency.
    AV matmuls pop SKEW tiles after their score."""

    def pop_av(self):
        slot, rl, i, ot, g, q = self.avq.popleft()
        _av_half(self, q, slot, rl, i, ot, g)
        if i == len(rl) - 1:
            _norm(self, q, slot, ot)

    def pop_dense(self):
        """Run one dense unit. An outproj unit of phase q reads otn columns
        written by phase q's norms, so every pending AV of phase <= q must be
        emitted first (emission order IS dependency order for the tile
        framework: a read emitted before its writer reads stale data)."""
        kind, qu, run = self.dense[0]
        if kind == "op" and self.avq and self.avq[0][5] <= qu:
            self.pop_av()
            return
        self.dense.popleft()
        run()

    def drain_av(self):
        while self.avq:
            self.pop_av()

    def drain_dense(self):
        while self.dense:
            self.pop_dense()


def _body(nc, tc, ctx, xT, wqkvT, woT, qrow, btbl, part):
    k = _K()
    k.nc = nc
    k.part = part
    k.xT = xT
    k.avq = deque()
    k.dense = deque()

    const = ctx.enter_context(tc.tile_pool(name="const", bufs=1))
    k.xtp = ctx.enter_context(tc.tile_pool(name="xt", bufs=2))
    k.stg = ctx.enter_context(tc.tile_pool(name="stg", bufs=3))
    k.gp = ctx.enter_context(tc.tile_pool(name="g", bufs=10))
    k.rcp = ctx.enter_context(tc.tile_pool(name="rc", bufs=2))
    k.bcp = ctx.enter_context(tc.tile_pool(name="bc", bufs=2))
    k.osp = ctx.enter_context(tc.tile_pool(name="ostage", bufs=4))

    # ---------- resident constants ----------------------------------------
    k.wqkv_res = const.tile([128, NE, ES + 2 * D], BF16)
    k.wo_res = const.tile([128, 2, E], BF16)
    k.qTa = []
    for j in range(HL):
        qa = const.tile([65, T], BF16, tag=f"qTa{j}")
        k.qTa.append(qa)
    k.kTa = const.tile([65, T], BF16)
    k.v_aug = const.tile([128, NT128, D + 1], BF16)
    k.btbl_t = const.tile([128, HL * NT128], F32)
    k.ident = const.tile([128, 128], BF16)
    k.otn = []
    for p2 in range(2):
        o = const.tile([128, T], BF16, tag=f"otn{p2}")
        k.otn.append(o)

    # ---------- 8 PSUM banks: (acc|po) 2 + st 4 + ot 2 --------------------
    k.pup = ctx.enter_context(tc.tile_pool(name="ps_acc", bufs=2, space="PSUM"))
    k.stp = ctx.enter_context(tc.tile_pool(name="st_ps", bufs=4, space="PSUM"))
    k.otp = ctx.enter_context(tc.tile_pool(name="ot_ps", bufs=2, space="PSUM"))

    def wdma(pl):  # weight loads, interleaved chunk-by-chunk with x at q0
        if pl == 0:
            nc.sync.dma_start(
                out=k.wqkv_res[:, 0:1, :],
                in_=bass.AP(tensor=wqkvT, offset=0,
                            ap=[[384, 128], [1, 384]]))
        elif pl == 1:
            nc.sync.dma_start(
                out=k.wqkv_res[:, 1:8, :],
                in_=bass.AP(tensor=wqkvT, offset=128 * 384,
                            ap=[[384, 128], [128 * 384, 7], [1, 384]]))
        elif pl == 2:
            nc.sync.dma_start(
                out=k.wqkv_res[:, 8:16, :],
                in_=bass.AP(tensor=wqkvT, offset=8 * 128 * 384,
                            ap=[[384, 128], [128 * 384, 8], [1, 384]]))
        elif pl == 3:
            # off the SP queue: small constants via the scalar engine
            for j in range(HL):
                nc.scalar.dma_start(out=k.qTa[j][64:65, :],
                                    in_=qrow[j:j + 1, :])
            nc.scalar.dma_start(out=k.btbl_t, in_=btbl[:, :])
            nc.gpsimd.memset(k.kTa[64:65, :], 1.0)
            nc.gpsimd.memset(k.v_aug[:, :, D:D + 1], 1.0)
            make_identity(nc, k.ident)
        elif pl == 4:  # Wo: first needed by outproj(0) units mid-phase 1
            nc.sync.dma_start(
                out=k.wo_res,
                in_=bass.AP(tensor=woT, offset=0,
                            ap=[[E, 128], [128 * E, 2], [1, E]]))
    k.wdma = wdma

    # bootstrap: phase 0 kv + group 0 emitted straight (DMA-paced); group 1
    # becomes dense filler so attention on slot pair (0,1) starts early
    _xt_dma(k, 0)
    steps0 = _proj_steps(k, 0)
    for step in steps0[:12]:       # kv chain+copy, transposes, g0 chain+copy
        step()
    k.dense.extend(("proj", 0, s) for s in steps0[12:])
    for q in range(NTQ):
        if q < NTQ - 1:
            _xt_dma(k, q + 1)
            ps = [("proj", q + 1, s) for s in _proj_steps(k, q + 1)]
            ops = list(k.dense)            # outproj units of q-1
            k.dense.clear()
            while ps or ops:               # round-robin merge
                if ops:
                    k.dense.append(ops.pop(0))
                if ps:
                    k.dense.append(ps.pop(0))
        _attn(k, q)
        k.drain_dense()            # any proj steps attn didn't absorb
        if q < NTQ - 1:
            _push_outproj(k, q)
    _tail_outproj(k)


def _xt_dma(k, q):
    """x column-slice load for phase q; first-e chunks split for fast start."""
    nc = k.nc
    cs = q * TQ
    xt = k.xtp.tile([128, NE, TQ], BF16, tag="xt", name=f"xt{q}")
    k.xt_cur = xt
    if q == 0:
        k.wdma(0)
        nc.sync.dma_start(
            out=xt[:, 0:1, :],
            in_=bass.AP(tensor=k.xT, offset=cs, ap=[[T, 128], [1, TQ]]))
        k.wdma(1)
        nc.sync.dma_start(
            out=xt[:, 1:8, :],
            in_=bass.AP(tensor=k.xT, offset=128 * T + cs,
                        ap=[[T, 128], [128 * T, 7], [1, TQ]]))
        k.wdma(2)
        nc.sync.dma_start(
            out=xt[:, 8:16, :],
            in_=bass.AP(tensor=k.xT, offset=8 * 128 * T + cs,
                        ap=[[T, 128], [128 * T, 8], [1, TQ]]))
        k.wdma(3)
    else:
        if q == 1:
            k.wdma(4)
        for pl in range(2):
            nc.sync.dma_start(
                out=xt[:, 8 * pl:8 * pl + 8, :],
                in_=bass.AP(tensor=k.xT, offset=(8 * pl * 128) * T + cs,
                            ap=[[T, 128], [128 * T, 8], [1, TQ]]))


def _proj_steps(k, q):
    """Projection for phase q as a list of dense-work closures: matmul
    bundles of 4 e-chunks, copy steps, and the v transposes."""
    nc = k.nc
    cs = q * TQ
    xt = k.xt_cur
    steps = []
    state = {}

    def chain_step(grp, e0):        # 4 accumulation matmuls
        def run():
            if e0 == 0:
                c0, c1 = grp * 128, (grp + 1) * 128
                if grp == 2:
                    c0, c1 = ES, ES + 2 * D
                state[grp] = (k.pup.tile([128, TQ], F32, tag="ps",
                                         name=f"acc{q}_{grp}"), c0, c1)
            acc, c0, c1 = state[grp]
            for e in range(e0, e0 + 4):
                nc.tensor.matmul(acc, k.wqkv_res[:, e, c0:c1], xt[:, e, :],
                                 start=(e == 0), stop=(e == NE - 1))
        return run

    def qcopy(grp):                 # split PSUM drain: ACT low / DVE shifted
        def run():
            acc = state[grp][0]
            nc.scalar.copy(out=k.qTa[2 * grp][0:64, cs:cs + TQ],
                           in_=acc[0:64, :])
            nc.vector.tensor_copy(out=k.qTa[2 * grp + 1][0:64, cs:cs + TQ],
                                  in_=acc[64:128, :])
        return run

    def kvcopy():
        def run():
            acc = state[2][0]
            nc.vector.tensor_copy(out=k.kTa[0:64, cs:cs + TQ],
                                  in_=acc[0:64, :])
            stv = k.stg.tile([128, TQ], BF16, tag="stg", name=f"stv{q}")
            nc.scalar.copy(out=stv[64:128, :], in_=acc[64:128, :])
            state["stv"] = stv
        return run

    def vtrans(mm):                 # v transpose via PE: [64,128] -> [128,64]
        def run():
            stv = state["stv"]
            tr = k.stp.tile([128, TQ], BF16, tag="st", name=f"tr{q}_{mm}")
            nc.tensor.transpose(tr[:, 0:D],
                                stv[64:128, mm * 128:(mm + 1) * 128],
                                k.ident[64:128, 64:128])
            nc.vector.tensor_copy(out=k.v_aug[:, 4 * q + mm, 0:D],
                                  in_=tr[:, 0:D])
        return run

    # kv first: phase 0 runs [kv, transposes, grp0] inline so attention on
    # slot pair (0,1) can start while grp1 is still DMA-paced.
    for grp in (2, 0, 1):
        for e0 in range(0, NE, 4):
            steps.append(chain_step(grp, e0))
        steps.append(qcopy(grp) if grp < 2 else kvcopy())
        if grp == 2:
            for mm in range(0, 4, 2):
                steps.append(lambda mm=mm: (vtrans(mm)(), vtrans(mm + 1)()))
    return steps


def _score_half(k, q, slot, rl, ti):
    """Score matmul + exp + causal mask for one tile; returns the g tile."""
    nc = k.nc
    cs = q * TQ
    tk, lo, hi = rl[ti]
    st = k.stp.tile([128, TQ], F32, tag="st")
    nc.tensor.matmul(st[:, lo - cs:hi - cs],
                     k.kTa[:, tk * 128:(tk + 1) * 128],
                     k.qTa[slot][:, lo:hi], start=True, stop=True)
    g = k.gp.tile([128, TQ], BF16, tag="g")
    if ti == 0 and hi - lo < TQ:
        nc.gpsimd.memset(g, 0.0)
    nc.scalar.activation(
        out=g[:, lo - cs:hi - cs], in_=st[:, lo - cs:hi - cs], func=EXP,
        bias=k.btbl_t[:, slot * NT128 + tk:slot * NT128 + tk + 1], scale=1.0)
    if tk >= 4 * q:  # diagonal block: causal mask, keep j <= i
        d0 = tk * 128 - cs
        nc.gpsimd.affine_select(
            out=g[:, d0:d0 + 128], in_=g[:, d0:d0 + 128],
            compare_op=mybir.AluOpType.is_ge,
            fill=0.0, base=0, pattern=[[1, 128]], channel_multiplier=-1)
    return g


def _av_half(k, q, slot, rl, ti, ot, g):
    """Accumulate one tile's g @ v into the head-output PSUM."""
    nc = k.nc
    cs = q * TQ
    tk, lo, hi = rl[ti]
    a_lo, a_hi = (0, TQ) if ti == 0 else (lo - cs, hi - cs)
    nc.tensor.matmul(ot[:, a_lo:a_hi], k.v_aug[:, tk, :], g[:, a_lo:a_hi],
                     start=(ti == 0), stop=(ti == len(rl) - 1))


def _norm(k, q, slot, ot):
    """headout = ot[0:64] / ot[64]; write into otn pair layout. The
    denominator row sits at PSUM partition 64; DVE handles the shifted
    reciprocal to partition 0 and the shifted odd-half multiply directly
    (verified on HW), so no staging DMAs are needed."""
    nc = k.nc
    cs, ce = q * TQ, (q + 1) * TQ
    rc = k.rcp.tile([1, TQ], F32, tag="rc")
    nc.vector.reciprocal(out=rc[0:1, :], in_=ot[64:65, :])
    bc = k.bcp.tile([64, TQ], F32, tag="bc")
    nc.gpsimd.partition_broadcast(bc, rc[0:1, :])
    pair, half = slot // 2, slot % 2
    nc.vector.tensor_mul(out=k.otn[pair][half * 64:half * 64 + 64, cs:ce],
                         in0=ot[0:64, :], in1=bc)


def _attn(k, q):
    """Attention for all 4 slots, pairwise interleaved, beat-scheduled:
    each beat emits one score tile, pops due AVs (SKEW behind), and pops
    dense work at a rate that exhausts the dense queue with the tiles."""
    seq = []
    for sA, sB in ((0, 1), (2, 3)):
        rlA, rlB = _tk_ranges(k_q := q, WS[sA]), _tk_ranges(q, WS[sB])
        otA = k.otp.tile([65, TQ], F32, tag="ot", name=f"ot{q}_{sA}")
        otB = k.otp.tile([65, TQ], F32, tag="ot", name=f"ot{q}_{sB}")
        for i in range(max(len(rlA), len(rlB))):
            if i < len(rlA):
                seq.append((sA, rlA, i, otA))
            if i < len(rlB):
                seq.append((sB, rlB, i, otB))
    for n, (slot, rl, i, ot) in enumerate(seq):
        g = _score_half(k, q, slot, rl, i)
        k.avq.append((slot, rl, i, ot, g, q))
        if len(k.avq) > SKEW:
            k.pop_av()
        left = len(seq) - n - 1
        ndense = len(k.dense) if left == 0 else (len(k.dense) + left - 1) // left
        for _ in range(min(ndense, 3 if left else len(k.dense))):
            if k.dense:
                k.pop_dense()


def _tail_outproj(k):
    """Last phase's output projection. otn[0] (slot pair 0,1) is final
    before the last AV drain, so those half-matmuls preheat PSUM banks while
    the drain's norm chains run; otn[1] halves, copies, and split DMAs
    follow. Keeps the PE fed through the very end."""
    nc = k.nc
    t0 = 4 * (NTQ - 1)
    obs = {}

    def ensure_ob(t):
        if t not in obs:
            obs[t] = k.osp.tile([128, 4, TQ], BF16, tag="ob", name=f"tob{t}")
        return obs[t]

    def a_half(t, o):
        pool, tag = (k.pup, "ps") if (t + o) % 2 == 0 else (k.stp, "st")
        po = pool.tile([128, TQ], F32, tag=tag, name=f"tpo{t}_{o}")
        nc.tensor.matmul(po, k.otn[0][:, t * 128:(t + 1) * 128],
                         k.wo_res[:, 0, o * TQ:(o + 1) * TQ],
                         start=True, stop=False)
        return po

    def finish(t, o, po):
        nc.tensor.matmul(po, k.otn[1][:, t * 128:(t + 1) * 128],
                         k.wo_res[:, 1, o * TQ:(o + 1) * TQ],
                         start=False, stop=True)
        ob = ensure_ob(t)
        if o % 2:
            nc.scalar.copy(out=ob[:, o, :], in_=po)
        else:
            nc.vector.tensor_copy(out=ob[:, o, :], in_=po)
        if o == 1:
            nc.sync.dma_start(out=k.part[t * 128:(t + 1) * 128, 0:2 * TQ],
                              in_=ob[:, 0:2, :])
        elif o == 3:
            nc.sync.dma_start(out=k.part[t * 128:(t + 1) * 128, 2 * TQ:4 * TQ],
                              in_=ob[:, 2:4, :])

    pre = [(t0, 0), (t0, 1), (t0 + 1, 0), (t0 + 1, 1), (t0 + 2, 0), (t0 + 2, 1)]
    pos = {}
    for t, o in pre:
        pos[(t, o)] = a_half(t, o)
        for _ in range(2):
            if k.avq:
                k.pop_av()
    k.drain_av()
    for t, o in pre:
        finish(t, o, pos[(t, o)])
    rest = [(t0 + 3, 0), (t0 + 3, 1), (t0, 2), (t0, 3), (t0 + 1, 2),
            (t0 + 1, 3), (t0 + 2, 2), (t0 + 2, 3), (t0 + 3, 2), (t0 + 3, 3)]
    for t, o in rest:
        finish(t, o, a_half(t, o))


def _push_outproj(k, q, tail=False):
    """Queue output projection for phase q's 4 token blocks as dense units.
    In the tail (no attention left to hide behind), po tiles alternate
    between the two PSUM pools so copies never gate the next matmul."""
    nc = k.nc
    state = {}

    def unit(t, o, use_st):
        def run():
            if o == 0:
                state[t] = k.osp.tile([128, 4, TQ], BF16, tag="ob",
                                      name=f"ob{t}")
            ob = state[t]
            pool, tag = (k.stp, "st") if use_st else (k.pup, "ps")
            po = pool.tile([128, TQ], F32, tag=tag, name=f"po{t}_{o}")
            nc.tensor.matmul(po, k.otn[0][:, t * 128:(t + 1) * 128],
                             k.wo_res[:, 0, o * TQ:(o + 1) * TQ],
                             start=True, stop=False)
            nc.tensor.matmul(po, k.otn[1][:, t * 128:(t + 1) * 128],
                             k.wo_res[:, 1, o * TQ:(o + 1) * TQ],
                             start=False, stop=True)
            if o % 2:
                nc.scalar.copy(out=ob[:, o, :], in_=po)
            else:
                nc.vector.tensor_copy(out=ob[:, o, :], in_=po)
            if tail:  # halve drain latency: ship each ob half as it fills
                if o == 1:
                    nc.sync.dma_start(
                        out=k.part[t * 128:(t + 1) * 128, 0:2 * TQ],
                        in_=ob[:, 0:2, :])
                elif o == 3:
                    nc.sync.dma_start(
                        out=k.part[t * 128:(t + 1) * 128, 2 * TQ:4 * TQ],
                        in_=ob[:, 2:4, :])
            elif o == 3:
                nc.sync.dma_start(out=k.part[t * 128:(t + 1) * 128, :],
                                  in_=ob[:, :, :])
        return run

    n = 0
    for t in range(4 * q, 4 * q + 4):
        for o in range(4):
            k.dense.append(("op", q, unit(t, o, tail and (n % 2 == 1))))
            n += 1


def _prepare_in_maps(x, Wq, Wk, Wv, Wo):
    xTn = np.ascontiguousarray(x[0].T).astype(NPBF16)
    scale = np.float64(D) ** -0.5
    i = np.arange(T, dtype=np.float64)
    p = np.arange(128, dtype=np.float64)
    kk = np.arange(NT128, dtype=np.float64)
    in_maps = []
    for c in range(NCORES):
        hs = [24 + c, c, 16 + c, 8 + c]   # window profile WS = [16, 1, 10, 3]
        wq_rows = np.concatenate(
            [Wq[h * D:(h + 1) * D, :] * scale for h in hs], axis=0)  # [256, E]
        wkv = np.concatenate([Wk, Wv], axis=0)                       # [128, E]
        wqkvT = np.ascontiguousarray(
            np.concatenate([wq_rows, wkv], axis=0).T).astype(NPBF16)
        woT = np.ascontiguousarray(
            np.concatenate([Wo[:, h * D:(h + 1) * D] for h in hs], axis=1).T
        ).astype(NPBF16)
        slopes = np.power(2.0, -8.0 * (np.asarray(hs, np.float64) + 1.0) / H)
        qrow_n = (-slopes[:, None] * i[None, :]).astype(NPBF16)      # [HL, T]
        btbl_n = (slopes[:, None, None] * (kk[None, :, None] * 128 + p[None, None, :]))
        btbl_n = np.ascontiguousarray(
            btbl_n.transpose(2, 0, 1).reshape(128, HL * NT128)).astype(np.float32)
        in_maps.append({
            "xT": xTn, "wqkvT": wqkvT, "woT": woT,
            "qrow": qrow_n, "btbl": btbl_n,
        })
    return in_maps


def kernel(x, Wq, Wk, Wv, Wo, attention_mask, _trace=False, _trace_cores=None):
    x = np.asarray(x, dtype=np.float32)
    Wq = np.asarray(Wq, dtype=np.float32)
    Wk = np.asarray(Wk, dtype=np.float32)
    Wv = np.asarray(Wv, dtype=np.float32)
    Wo = np.asarray(Wo, dtype=np.float32)

    if "nc" not in _CACHE:
        _CACHE["nc"] = _build_nc()
    nc = _CACHE["nc"]

    in_maps = _prepare_in_maps(x, Wq, Wk, Wv, Wo)
    kwargs = {}
    if _trace:
        kwargs = {"trace": True, "trace_cores": _trace_cores or [0]}
    res = run_bass_kernel_spmd(nc, in_maps, core_ids=list(range(NCORES)), **kwargs)
    acc = np.zeros((T, E), dtype=np.float64)
    for r in res.results:
        acc += np.asarray(r["part"]).astype(np.float64)
    out = acc.astype(np.float32)[None, :, :]
    if _trace:
        _CACHE["last_result"] = res
    return out


# revision 4
# speedup vs baseline: 1.0136x; 1.0136x over previous
"""MQA attention (32 query heads, 1 KV head, ALiBi, causal) on 8 trn2 cores.

Sharding: tensor-parallel over query heads (4 heads/core). Wq rows and Wo
columns are sharded; x, Wk, Wv are replicated. Each core computes a partial
[T, E] output (its 4 heads pushed through its Wo column-shard); the host sums
the 8 partials.

v2 design vs baseline:
- bf16 operands everywhere on the PE (1 cyc/col at any width, no fp32r
  256-col padding); fp32 accumulation in PSUM; partial output written bf16.
- ALiBi-windowed attention: head h's weights decay as exp(-s_h * dist), so
  keys beyond dist tau/s_h contribute < e^-20 relative and are skipped at
  128-block granularity. Cores get head sets {24+c, 16+c, 8+c, c} with
  identical window profiles W = [16(full), 10, 3, 1] blocks, so the SPMD
  instruction stream is core-independent and load-balanced.
- Few large DMAs (HWDGE serializes at ~625ns/DMA): x loaded in big strided
  DMAs, weights in 4, output staged to [128, 2048] bf16 rows.
- The PE executes strictly in program order, so emission order IS the
  schedule: score->AV skew of 4 tiles, AV/outproj work deferred across
  phase boundaries as filler between other PE ops, so the PE never sits
  behind a parked instruction waiting on the ACT/Pool exp/mask chain.

Math per core c (slots j=0..3, heads hs=[24+c, 16+c, 8+c, c]):
  qT_j = (Wq_hj * D^-0.5) @ x^T                    [64, T]
  kT   = Wk @ x^T, v = (Wv @ x^T)^T                [64, T], [T, 64]
  ST[j, i] = kT[:,j].q  +  (-s_h * i)              via augmented row (ones in
                                                   kTa row 64, -s_h*i in qTa)
  g = exp(ST + s_h*j)      (per-partition ACT bias; per-column factors cancel
                            in softmax normalization; causal mask via
                            affine_select on diagonal blocks; key blocks
                            outside the ALiBi window skipped)
  OT_aug = [v | 1]^T @ g   -> rows 0:64 = unnormalized head out^T,
                              row 64    = softmax denominator
  headout^T = OT / denom   (partition_broadcast of 1/denom)
  partial = headout^T.T @ WoT_shard                [T, E] bf16
"""

from collections import deque

import numpy as np
import ml_dtypes

import concourse.bacc as bacc
import concourse.bass as bass
import concourse.mybir as mybir
import concourse.tile as tile
from concourse.masks import make_identity
from concourse.bass_utils import run_bass_kernel_spmd

T = 2048          # tokens
E = 2048          # embed dim
H = 32            # query heads
D = 64            # head dim
NCORES = 8
HL = H // NCORES  # 4 heads per core
ES = HL * D       # 256 = per-core E shard
TQ = 512          # query-phase tile
NTQ = T // TQ     # 4
NE = E // 128     # 16 contraction chunks
NT128 = T // 128  # 16

WS = [16, 1, 10, 3]   # per-slot ALiBi windows (key blocks beyond diagonal);
                      # pairs (0,1) and (2,3) balance long+short tile lists
SKEW = 5              # score -> AV deferral depth (tiles)

F32 = mybir.dt.float32
BF16 = mybir.dt.bfloat16
EXP = mybir.ActivationFunctionType.Exp
NPBF16 = ml_dtypes.bfloat16

_CACHE = {}


def _tk_ranges(q, W):
    """(tk, lo, hi) global-column score tiles for query block q, window W."""
    cs, ce = q * TQ, (q + 1) * TQ
    out = []
    for tk in range(max(0, 4 * q - W), 4 * q + 4):
        lo = max(cs, tk * 128)
        hi = min(ce, (tk + W + 1) * 128)
        out.append((tk, lo, hi))
    return out


def _build_nc():
    nc = bacc.Bacc("TRN2")
    xT = nc.dram_tensor("xT", [E, T], BF16, kind="ExternalInput")
    wqkvT = nc.dram_tensor("wqkvT", [E, ES + 2 * D], BF16, kind="ExternalInput")
    woT = nc.dram_tensor("woT", [ES, E], BF16, kind="ExternalInput")
    qrow = nc.dram_tensor("qrow", [HL, T], BF16, kind="ExternalInput")
    btbl = nc.dram_tensor("btbl", [128, HL * NT128], F32, kind="ExternalInput")
    part = nc.dram_tensor("part", [T, E], BF16, kind="ExternalOutput")

    from contextlib import ExitStack
    with tile.TileContext(nc) as tc, ExitStack() as ctx:
        _body(nc, tc, ctx, xT, wqkvT, woT, qrow, btbl, part)
    nc.finalize()
    return nc


class _K:
    """Kernel emission state: tile pools, resident tiles, and the deferral
    queues. The PE runs strictly in program order, so emission order is the
    schedule: attention tiles (whose g comes back through the ACT/Pool
    exp/mask chain) are interleaved beat-by-beat with "dense" PE work
    (projection chains, output projection) that has no cross-engine latency.
    AV matmuls pop SKEW tiles after their score."""

    def pop_av(self):
        slot, rl, i, ot, g, q = self.avq.popleft()
        _av_half(self, q, slot, rl, i, ot, g)
        if i == len(rl) - 1:
            _norm(self, q, slot, ot)

    def pop_dense(self):
        """Run one dense unit. An outproj unit of phase q reads otn columns
        written by phase q's norms, so every pending AV of phase <= q must be
        emitted first (emission order IS dependency order for the tile
        framework: a read emitted before its writer reads stale data)."""
        kind, qu, run = self.dense[0]
        if kind == "op" and self.avq and self.avq[0][5] <= qu:
            self.pop_av()
            return
        self.dense.popleft()
        run()

    def drain_av(self):
        while self.avq:
            self.pop_av()

    def drain_dense(self):
        while self.dense:
            self.pop_dense()


def _body(nc, tc, ctx, xT, wqkvT, woT, qrow, btbl, part):
    k = _K()
    k.nc = nc
    k.part = part
    k.xT = xT
    k.avq = deque()
    k.dense = deque()

    const = ctx.enter_context(tc.tile_pool(name="const", bufs=1))
    k.xtp = ctx.enter_context(tc.tile_pool(name="xt", bufs=2))
    k.stg = ctx.enter_context(tc.tile_pool(name="stg", bufs=3))
    k.gp = ctx.enter_context(tc.tile_pool(name="g", bufs=8))
    k.rcp = ctx.enter_context(tc.tile_pool(name="rc", bufs=3))
    k.bcp = ctx.enter_context(tc.tile_pool(name="bc", bufs=3))
    k.osp = ctx.enter_context(tc.tile_pool(name="ostage", bufs=4))

    # ---------- resident constants ----------------------------------------
    k.wqkv_res = const.tile([128, NE, ES + 2 * D], BF16)
    k.wo_res = const.tile([128, 2, E], BF16)
    k.qTa = []
    for j in range(HL):
        qa = const.tile([65, T], BF16, tag=f"qTa{j}")
        k.qTa.append(qa)
    k.kTa = const.tile([65, T], BF16)
    k.v_aug = const.tile([128, NT128, D + 1], BF16)
    k.btbl_t = const.tile([128, HL * NT128], F32)
    k.ident = const.tile([128, 128], BF16)
    k.otn = []
    for p2 in range(2):
        o = const.tile([128, T], BF16, tag=f"otn{p2}")
        k.otn.append(o)

    # ---------- 8 PSUM banks: (acc|po) 2 + st 4 + ot 2 --------------------
    k.pup = ctx.enter_context(tc.tile_pool(name="ps_acc", bufs=2, space="PSUM"))
    k.stp = ctx.enter_context(tc.tile_pool(name="st_ps", bufs=4, space="PSUM"))
    k.otp = ctx.enter_context(tc.tile_pool(name="ot_ps", bufs=2, space="PSUM"))

    def wdma(pl):  # weight loads, interleaved chunk-by-chunk with x at q0
        if pl == 0:
            nc.sync.dma_start(
                out=k.wqkv_res[:, 0:1, :],
                in_=bass.AP(tensor=wqkvT, offset=0,
                            ap=[[384, 128], [1, 384]]))
        elif pl in (1, 2, 5, 6):
            a, b = {1: (1, 4), 2: (4, 8), 5: (8, 12), 6: (12, 16)}[pl]
            nc.sync.dma_start(
                out=k.wqkv_res[:, a:b, :],
                in_=bass.AP(tensor=wqkvT, offset=a * 128 * 384,
                            ap=[[384, 128], [128 * 384, b - a], [1, 384]]))
        elif pl == 3:
            # off the SP queue: small constants via the scalar engine
            for j in range(HL):
                nc.scalar.dma_start(out=k.qTa[j][64:65, :],
                                    in_=qrow[j:j + 1, :])
            nc.scalar.dma_start(out=k.btbl_t, in_=btbl[:, :])
            nc.gpsimd.memset(k.kTa[64:65, :], 1.0)
            nc.gpsimd.memset(k.v_aug[:, :, D:D + 1], 1.0)
            make_identity(nc, k.ident)
        elif pl == 4:  # Wo: first needed by outproj(0) units mid-phase 1
            nc.sync.dma_start(
                out=k.wo_res,
                in_=bass.AP(tensor=woT, offset=0,
                            ap=[[E, 128], [128 * E, 2], [1, E]]))
    k.wdma = wdma

    # bootstrap: phase 0 kv + group 0 emitted straight (DMA-paced); group 1
    # becomes dense filler so attention on slot pair (0,1) starts early
    _xt_dma(k, 0)
    steps0 = _proj_steps(k, 0)
    for step in steps0[:12]:       # kv chain+copy, transposes, g0 chain+copy
        step()
    k.dense.extend(("proj", 0, s) for s in steps0[12:])
    for q in range(NTQ):
        if q < NTQ - 1:
            _xt_dma(k, q + 1)
            ps = [("proj", q + 1, s) for s in _proj_steps(k, q + 1)]
            ops = list(k.dense)            # outproj units of q-1
            k.dense.clear()
            while ps or ops:               # round-robin merge
                if ops:
                    k.dense.append(ops.pop(0))
                if ps:
                    k.dense.append(ps.pop(0))
        _attn(k, q)
        k.drain_dense()            # any proj steps attn didn't absorb
        if q < NTQ - 1:
            _push_outproj(k, q)
    _tail_outproj(k)


def _xt_dma(k, q):
    """x column-slice load for phase q; first-e chunks split for fast start."""
    nc = k.nc
    cs = q * TQ
    xt = k.xtp.tile([128, NE, TQ], BF16, tag="xt", name=f"xt{q}")
    k.xt_cur = xt
    if q == 0:
        k.wdma(0)
        nc.sync.dma_start(
            out=xt[:, 0:1, :],
            in_=bass.AP(tensor=k.xT, offset=cs, ap=[[T, 128], [1, TQ]]))
        for pl, (a, b) in ((1, (1, 4)), (2, (4, 8)), (5, (8, 12)),
                           (6, (12, 16))):
            k.wdma(pl)
            nc.sync.dma_start(
                out=xt[:, a:b, :],
                in_=bass.AP(tensor=k.xT, offset=a * 128 * T + cs,
                            ap=[[T, 128], [128 * T, b - a], [1, TQ]]))
        k.wdma(3)
    else:
        nsp = 4 if q == 1 else 2
        for pl in range(nsp):
            w = NE // nsp
            nc.sync.dma_start(
                out=xt[:, w * pl:w * pl + w, :],
                in_=bass.AP(tensor=k.xT, offset=(w * pl * 128) * T + cs,
                            ap=[[T, 128], [128 * T, w], [1, TQ]]))
            if q == 1 and pl == 0:
                k.wdma(4)


def _proj_steps(k, q):
    """Projection for phase q as a list of dense-work closures: matmul
    bundles of 4 e-chunks, copy steps, and the v transposes."""
    nc = k.nc
    cs = q * TQ
    xt = k.xt_cur
    steps = []
    state = {}

    def chain_step(grp, e0):        # 4 accumulation matmuls
        def run():
            if e0 == 0:
                c0, c1 = grp * 128, (grp + 1) * 128
                if grp == 2:
                    c0, c1 = ES, ES + 2 * D
                state[grp] = (k.pup.tile([128, TQ], F32, tag="ps",
                                         name=f"acc{q}_{grp}"), c0, c1)
            acc, c0, c1 = state[grp]
            for e in range(e0, e0 + 4):
                nc.tensor.matmul(acc, k.wqkv_res[:, e, c0:c1], xt[:, e, :],
                                 start=(e == 0), stop=(e == NE - 1))
        return run

    def qcopy(grp):                 # split PSUM drain: ACT low / DVE shifted
        def run():
            acc = state[grp][0]
            nc.scalar.copy(out=k.qTa[2 * grp][0:64, cs:cs + TQ],
                           in_=acc[0:64, :])
            nc.vector.tensor_copy(out=k.qTa[2 * grp + 1][0:64, cs:cs + TQ],
                                  in_=acc[64:128, :])
        return run

    def kvcopy():
        def run():
            acc = state[2][0]
            nc.vector.tensor_copy(out=k.kTa[0:64, cs:cs + TQ],
                                  in_=acc[0:64, :])
            stv = k.stg.tile([128, TQ], BF16, tag="stg", name=f"stv{q}")
            nc.scalar.copy(out=stv[64:128, :], in_=acc[64:128, :])
            state["stv"] = stv
        return run

    def vtrans(mm):                 # v transpose via PE: [64,128] -> [128,64]
        def run():
            stv = state["stv"]
            tr = k.stp.tile([128, TQ], BF16, tag="st", name=f"tr{q}_{mm}")
            nc.tensor.transpose(tr[:, 0:D],
                                stv[64:128, mm * 128:(mm + 1) * 128],
                                k.ident[64:128, 64:128])
            nc.vector.tensor_copy(out=k.v_aug[:, 4 * q + mm, 0:D],
                                  in_=tr[:, 0:D])
        return run

    # kv first: phase 0 runs [kv, transposes, grp0] inline so attention on
    # slot pair (0,1) can start while grp1 is still DMA-paced.
    for grp in (2, 0, 1):
        for e0 in range(0, NE, 4):
            steps.append(chain_step(grp, e0))
        steps.append(qcopy(grp) if grp < 2 else kvcopy())
        if grp == 2:
            for mm in range(0, 4, 2):
                steps.append(lambda mm=mm: (vtrans(mm)(), vtrans(mm + 1)()))
    return steps


def _score_half(k, q, slot, rl, ti):
    """Score matmul + exp + causal mask for one tile; returns the g tile."""
    nc = k.nc
    cs = q * TQ
    tk, lo, hi = rl[ti]
    st = k.stp.tile([128, TQ], F32, tag="st")
    nc.tensor.matmul(st[:, lo - cs:hi - cs],
                     k.kTa[:, tk * 128:(tk + 1) * 128],
                     k.qTa[slot][:, lo:hi], start=True, stop=True)
    g = k.gp.tile([128, TQ], BF16, tag="g")
    if ti == 0 and hi - lo < TQ:
        nc.gpsimd.memset(g, 0.0)
    nc.scalar.activation(
        out=g[:, lo - cs:hi - cs], in_=st[:, lo - cs:hi - cs], func=EXP,
        bias=k.btbl_t[:, slot * NT128 + tk:slot * NT128 + tk + 1], scale=1.0)
    if tk >= 4 * q:  # diagonal block: causal mask, keep j <= i
        d0 = tk * 128 - cs
        nc.gpsimd.affine_select(
            out=g[:, d0:d0 + 128], in_=g[:, d0:d0 + 128],
            compare_op=mybir.AluOpType.is_ge,
            fill=0.0, base=0, pattern=[[1, 128]], channel_multiplier=-1)
    return g


def _av_half(k, q, slot, rl, ti, ot, g):
    """Accumulate one tile's g @ v into the head-output PSUM."""
    nc = k.nc
    cs = q * TQ
    tk, lo, hi = rl[ti]
    a_lo, a_hi = (0, TQ) if ti == 0 else (lo - cs, hi - cs)
    nc.tensor.matmul(ot[:, a_lo:a_hi], k.v_aug[:, tk, :], g[:, a_lo:a_hi],
                     start=(ti == 0), stop=(ti == len(rl) - 1))


def _norm(k, q, slot, ot):
    """headout = ot[0:64] / ot[64]; write into otn pair layout. The
    denominator row sits at PSUM partition 64; DVE handles the shifted
    reciprocal to partition 0 and the shifted odd-half multiply directly
    (verified on HW), so no staging DMAs are needed."""
    nc = k.nc
    cs, ce = q * TQ, (q + 1) * TQ
    rc = k.rcp.tile([1, TQ], F32, tag="rc")
    nc.vector.reciprocal(out=rc[0:1, :], in_=ot[64:65, :])
    bc = k.bcp.tile([64, TQ], F32, tag="bc")
    nc.gpsimd.partition_broadcast(bc, rc[0:1, :])
    pair, half = slot // 2, slot % 2
    nc.vector.tensor_mul(out=k.otn[pair][half * 64:half * 64 + 64, cs:ce],
                         in0=ot[0:64, :], in1=bc)


def _attn(k, q):
    """Attention for all 4 slots, pairwise interleaved, beat-scheduled:
    each beat emits one score tile, pops due AVs (SKEW behind), and pops
    dense work at a rate that exhausts the dense queue with the tiles."""
    seq = []
    for sA, sB in ((0, 1), (2, 3)):
        rlA, rlB = _tk_ranges(k_q := q, WS[sA]), _tk_ranges(q, WS[sB])
        otA = k.otp.tile([65, TQ], F32, tag="ot", name=f"ot{q}_{sA}")
        otB = k.otp.tile([65, TQ], F32, tag="ot", name=f"ot{q}_{sB}")
        for i in range(max(len(rlA), len(rlB))):
            if i < len(rlA):
                seq.append((sA, rlA, i, otA))
            if i < len(rlB):
                seq.append((sB, rlB, i, otB))
    for n, (slot, rl, i, ot) in enumerate(seq):
        g = _score_half(k, q, slot, rl, i)
        k.avq.append((slot, rl, i, ot, g, q))
        if len(k.avq) > SKEW:
            k.pop_av()
        left = len(seq) - n - 1
        ndense = len(k.dense) if left == 0 else (len(k.dense) + left - 1) // left
        for _ in range(min(ndense, 3 if left else len(k.dense))):
            if k.dense:
                k.pop_dense()


def _tail_outproj(k):
    """Last phase's output projection. otn[0] (slot pair 0,1) is final
    before the last AV drain, so those half-matmuls preheat PSUM banks while
    the drain's norm chains run; otn[1] halves, copies, and split DMAs
    follow. Keeps the PE fed through the very end."""
    nc = k.nc
    t0 = 4 * (NTQ - 1)
    obs = {}

    def ensure_ob(t):
        if t not in obs:
            obs[t] = k.osp.tile([128, 4, TQ], BF16, tag="ob", name=f"tob{t}")
        return obs[t]

    def a_half(t, o):
        pool, tag = (k.pup, "ps") if (t + o) % 2 == 0 else (k.stp, "st")
        po = pool.tile([128, TQ], F32, tag=tag, name=f"tpo{t}_{o}")
        nc.tensor.matmul(po, k.otn[0][:, t * 128:(t + 1) * 128],
                         k.wo_res[:, 0, o * TQ:(o + 1) * TQ],
                         start=True, stop=False)
        return po

    def finish(t, o, po):
        nc.tensor.matmul(po, k.otn[1][:, t * 128:(t + 1) * 128],
                         k.wo_res[:, 1, o * TQ:(o + 1) * TQ],
                         start=False, stop=True)
        ob = ensure_ob(t)
        if o % 2:
            nc.scalar.copy(out=ob[:, o, :], in_=po)
        else:
            nc.vector.tensor_copy(out=ob[:, o, :], in_=po)
        if o == 1:
            nc.sync.dma_start(out=k.part[t * 128:(t + 1) * 128, 0:2 * TQ],
                              in_=ob[:, 0:2, :])
        elif o == 3:
            nc.sync.dma_start(out=k.part[t * 128:(t + 1) * 128, 2 * TQ:4 * TQ],
                              in_=ob[:, 2:4, :])

    pre = [(t0, 0), (t0, 1), (t0 + 1, 0), (t0 + 1, 1), (t0 + 2, 0), (t0 + 2, 1)]
    pos = {}
    for t, o in pre:
        pos[(t, o)] = a_half(t, o)
        for _ in range(2):
            if k.avq:
                k.pop_av()
    k.drain_av()
    for t, o in pre:
        finish(t, o, pos[(t, o)])
    rest = [(t0 + 3, 0), (t0 + 3, 1), (t0, 2), (t0, 3), (t0 + 1, 2),
            (t0 + 1, 3), (t0 + 2, 2), (t0 + 2, 3), (t0 + 3, 2), (t0 + 3, 3)]
    for t, o in rest:
        finish(t, o, a_half(t, o))


def _push_outproj(k, q, tail=False):
    """Queue output projection for phase q's 4 token blocks as dense units.
    In the tail (no attention left to hide behind), po tiles alternate
    between the two PSUM pools so copies never gate the next matmul."""
    nc = k.nc
    state = {}

    def unit(t, o, use_st):
        def run():
            if o == 0:
                state[t] = k.osp.tile([128, 4, TQ], BF16, tag="ob",
                                      name=f"ob{t}")
            ob = state[t]
            pool, tag = (k.stp, "st") if use_st else (k.pup, "ps")
            po = pool.tile([128, TQ], F32, tag=tag, name=f"po{t}_{o}")
            nc.tensor.matmul(po, k.otn[0][:, t * 128:(t + 1) * 128],
                             k.wo_res[:, 0, o * TQ:(o + 1) * TQ],
                             start=True, stop=False)
            nc.tensor.matmul(po, k.otn[1][:, t * 128:(t + 1) * 128],
                             k.wo_res[:, 1, o * TQ:(o + 1) * TQ],
                             start=False, stop=True)
            if o % 2:
                nc.scalar.copy(out=ob[:, o, :], in_=po)
            else:
                nc.vector.tensor_copy(out=ob[:, o, :], in_=po)
            if tail:  # halve drain latency: ship each ob half as it fills
                if o == 1:
                    nc.sync.dma_start(
                        out=k.part[t * 128:(t + 1) * 128, 0:2 * TQ],
                        in_=ob[:, 0:2, :])
                elif o == 3:
                    nc.sync.dma_start(
                        out=k.part[t * 128:(t + 1) * 128, 2 * TQ:4 * TQ],
                        in_=ob[:, 2:4, :])
            elif o == 3:
                nc.sync.dma_start(out=k.part[t * 128:(t + 1) * 128, :],
                                  in_=ob[:, :, :])
        return run

    n = 0
    for t in range(4 * q, 4 * q + 4):
        for o in range(4):
            k.dense.append(("op", q, unit(t, o, tail and (n % 2 == 1))))
            n += 1


def _prepare_in_maps(x, Wq, Wk, Wv, Wo):
    xTn = np.ascontiguousarray(x[0].T).astype(NPBF16)
    scale = np.float64(D) ** -0.5
    i = np.arange(T, dtype=np.float64)
    p = np.arange(128, dtype=np.float64)
    kk = np.arange(NT128, dtype=np.float64)
    in_maps = []
    for c in range(NCORES):
        hs = [24 + c, c, 16 + c, 8 + c]   # window profile WS = [16, 1, 10, 3]
        wq_rows = np.concatenate(
            [Wq[h * D:(h + 1) * D, :] * scale for h in hs], axis=0)  # [256, E]
        wkv = np.concatenate([Wk, Wv], axis=0)                       # [128, E]
        wqkvT = np.ascontiguousarray(
            np.concatenate([wq_rows, wkv], axis=0).T).astype(NPBF16)
        woT = np.ascontiguousarray(
            np.concatenate([Wo[:, h * D:(h + 1) * D] for h in hs], axis=1).T
        ).astype(NPBF16)
        slopes = np.power(2.0, -8.0 * (np.asarray(hs, np.float64) + 1.0) / H)
        qrow_n = (-slopes[:, None] * i[None, :]).astype(NPBF16)      # [HL, T]
        btbl_n = (slopes[:, None, None] * (kk[None, :, None] * 128 + p[None, None, :]))
        btbl_n = np.ascontiguousarray(
            btbl_n.transpose(2, 0, 1).reshape(128, HL * NT128)).astype(np.float32)
        in_maps.append({
            "xT": xTn, "wqkvT": wqkvT, "woT": woT,
            "qrow": qrow_n, "btbl": btbl_n,
        })
    return in_maps


def kernel(x, Wq, Wk, Wv, Wo, attention_mask, _trace=False, _trace_cores=None):
    x = np.asarray(x, dtype=np.float32)
    Wq = np.asarray(Wq, dtype=np.float32)
    Wk = np.asarray(Wk, dtype=np.float32)
    Wv = np.asarray(Wv, dtype=np.float32)
    Wo = np.asarray(Wo, dtype=np.float32)

    if "nc" not in _CACHE:
        _CACHE["nc"] = _build_nc()
    nc = _CACHE["nc"]

    in_maps = _prepare_in_maps(x, Wq, Wk, Wv, Wo)
    kwargs = {}
    if _trace:
        kwargs = {"trace": True, "trace_cores": _trace_cores or [0]}
    res = run_bass_kernel_spmd(nc, in_maps, core_ids=list(range(NCORES)), **kwargs)
    acc = np.zeros((T, E), dtype=np.float64)
    for r in res.results:
        acc += np.asarray(r["part"]).astype(np.float64)
    out = acc.astype(np.float32)[None, :, :]
    if _trace:
        _CACHE["last_result"] = res
    return out


# revision 5
# speedup vs baseline: 1.0335x; 1.0196x over previous
"""MQA attention (32 query heads, 1 KV head, ALiBi, causal) on 8 trn2 cores.

Sharding: tensor-parallel over query heads (4 heads/core). Wq rows and Wo
columns are sharded; x, Wk, Wv are replicated. Each core computes a partial
[T, E] output (its 4 heads pushed through its Wo column-shard); the host sums
the 8 partials.

v2 design vs baseline:
- bf16 operands everywhere on the PE (1 cyc/col at any width, no fp32r
  256-col padding); fp32 accumulation in PSUM; partial output written bf16.
- ALiBi-windowed attention: head h's weights decay as exp(-s_h * dist), so
  keys beyond dist tau/s_h contribute < e^-20 relative and are skipped at
  128-block granularity. Cores get head sets {24+c, 16+c, 8+c, c} with
  identical window profiles W = [16(full), 10, 3, 1] blocks, so the SPMD
  instruction stream is core-independent and load-balanced.
- Few large DMAs (HWDGE serializes at ~625ns/DMA): x loaded in big strided
  DMAs, weights in 4, output staged to [128, 2048] bf16 rows.
- The PE executes strictly in program order, so emission order IS the
  schedule: score->AV skew of 4 tiles, AV/outproj work deferred across
  phase boundaries as filler between other PE ops, so the PE never sits
  behind a parked instruction waiting on the ACT/Pool exp/mask chain.

Math per core c (slots j=0..3, heads hs=[24+c, 16+c, 8+c, c]):
  qT_j = (Wq_hj * D^-0.5) @ x^T                    [64, T]
  kT   = Wk @ x^T, v = (Wv @ x^T)^T                [64, T], [T, 64]
  ST[j, i] = kT[:,j].q  +  (-s_h * i)              via augmented row (ones in
                                                   kTa row 64, -s_h*i in qTa)
  g = exp(ST + s_h*j)      (per-partition ACT bias; per-column factors cancel
                            in softmax normalization; causal mask via
                            affine_select on diagonal blocks; key blocks
                            outside the ALiBi window skipped)
  OT_aug = [v | 1]^T @ g   -> rows 0:64 = unnormalized head out^T,
                              row 64    = softmax denominator
  headout^T = OT / denom   (partition_broadcast of 1/denom)
  partial = headout^T.T @ WoT_shard                [T, E] bf16
"""

from collections import deque

import numpy as np
import ml_dtypes

import concourse.bacc as bacc
import concourse.bass as bass
import concourse.mybir as mybir
import concourse.tile as tile
from concourse.masks import make_identity
from concourse.bass_utils import run_bass_kernel_spmd

T = 2048          # tokens
E = 2048          # embed dim
H = 32            # query heads
D = 64            # head dim
NCORES = 8
HL = H // NCORES  # 4 heads per core
ES = HL * D       # 256 = per-core E shard
TQ = 512          # query-phase tile
NTQ = T // TQ     # 4
NE = E // 128     # 16 contraction chunks
NT128 = T // 128  # 16

WS = [16, 1, 10, 3]   # per-slot ALiBi windows (key blocks beyond diagonal);
                      # pairs (0,1) and (2,3) balance long+short tile lists
SKEW = 5              # score -> AV deferral depth (tiles)

F32 = mybir.dt.float32
BF16 = mybir.dt.bfloat16
EXP = mybir.ActivationFunctionType.Exp
NPBF16 = ml_dtypes.bfloat16

_CACHE = {}


def _tk_ranges(q, W):
    """(tk, lo, hi) global-column score tiles for query block q, window W."""
    cs, ce = q * TQ, (q + 1) * TQ
    out = []
    for tk in range(max(0, 4 * q - W), 4 * q + 4):
        lo = max(cs, tk * 128)
        hi = min(ce, (tk + W + 1) * 128)
        out.append((tk, lo, hi))
    return out


def _build_nc():
    nc = bacc.Bacc("TRN2")
    xT = nc.dram_tensor("xT", [E, T], BF16, kind="ExternalInput")
    wqkvT = nc.dram_tensor("wqkvT", [E, ES + 2 * D], BF16, kind="ExternalInput")
    woT = nc.dram_tensor("woT", [ES, E], BF16, kind="ExternalInput")
    qrow = nc.dram_tensor("qrow", [HL, T], BF16, kind="ExternalInput")
    btbl = nc.dram_tensor("btbl", [128, HL * NT128], F32, kind="ExternalInput")
    part = nc.dram_tensor("part", [T, E], BF16, kind="ExternalOutput")

    from contextlib import ExitStack
    with tile.TileContext(nc) as tc, ExitStack() as ctx:
        _body(nc, tc, ctx, xT, wqkvT, woT, qrow, btbl, part)
    nc.finalize()
    return nc


class _K:
    """Kernel emission state: tile pools, resident tiles, and the deferral
    queues. The PE runs strictly in program order, so emission order is the
    schedule: attention tiles (whose g comes back through the ACT/Pool
    exp/mask chain) are interleaved beat-by-beat with "dense" PE work
    (projection chains, output projection) that has no cross-engine latency.
    AV matmuls pop SKEW tiles after their score."""

    def pop_av(self):
        slot, rl, i, ot, g, q = self.avq.popleft()
        _av_half(self, q, slot, rl, i, ot, g)
        if i == len(rl) - 1:
            _norm(self, q, slot, ot)

    def pop_dense(self):
        """Run one dense unit. An outproj unit of phase q reads otn columns
        written by phase q's norms, so every pending AV of phase <= q must be
        emitted first (emission order IS dependency order for the tile
        framework: a read emitted before its writer reads stale data)."""
        kind, qu, run = self.dense[0]
        if kind == "op" and self.avq and self.avq[0][5] <= qu:
            self.pop_av()
            return
        self.dense.popleft()
        run()

    def drain_av(self):
        while self.avq:
            self.pop_av()

    def drain_dense(self):
        while self.dense:
            self.pop_dense()


def _body(nc, tc, ctx, xT, wqkvT, woT, qrow, btbl, part):
    k = _K()
    k.nc = nc
    k.part = part
    k.xT = xT
    k.avq = deque()
    k.dense = deque()

    const = ctx.enter_context(tc.tile_pool(name="const", bufs=1))
    k.xtp = ctx.enter_context(tc.tile_pool(name="xt", bufs=2))
    k.stg = ctx.enter_context(tc.tile_pool(name="stg", bufs=3))
    k.gp = ctx.enter_context(tc.tile_pool(name="g", bufs=8))
    k.rcp = ctx.enter_context(tc.tile_pool(name="rc", bufs=3))
    k.bcp = ctx.enter_context(tc.tile_pool(name="bc", bufs=3))
    k.osp = ctx.enter_context(tc.tile_pool(name="ostage", bufs=4))

    # ---------- resident constants ----------------------------------------
    k.wqkv_res = const.tile([128, NE, ES + 2 * D], BF16)
    k.wo_res = const.tile([128, 2, E], BF16)
    k.qTa = []
    for j in range(HL):
        qa = const.tile([65, T], BF16, tag=f"qTa{j}")
        k.qTa.append(qa)
    k.kTa = const.tile([65, T], BF16)
    k.v_aug = const.tile([128, NT128, D + 1], BF16)
    k.btbl_t = const.tile([128, HL * NT128], F32)
    k.ident = const.tile([128, 128], BF16)
    k.otn = []
    for p2 in range(2):
        o = const.tile([128, T], BF16, tag=f"otn{p2}")
        k.otn.append(o)

    # ---------- 8 PSUM banks: (acc|po) 2 + st 4 + ot 2 --------------------
    k.pup = ctx.enter_context(tc.tile_pool(name="ps_acc", bufs=2, space="PSUM"))
    k.stp = ctx.enter_context(tc.tile_pool(name="st_ps", bufs=4, space="PSUM"))
    k.otp = ctx.enter_context(tc.tile_pool(name="ot_ps", bufs=2, space="PSUM"))

    def wdma(pl):  # weight loads, interleaved chunk-by-chunk with x at q0
        if pl == 0:
            nc.sync.dma_start(
                out=k.wqkv_res[:, 0:1, :],
                in_=bass.AP(tensor=wqkvT, offset=0,
                            ap=[[384, 128], [1, 384]]))
        elif pl in (1, 2, 5, 6):
            a, b = {1: (1, 4), 2: (4, 8), 5: (8, 12), 6: (12, 16)}[pl]
            nc.scalar.dma_start(
                out=k.wqkv_res[:, a:b, :],
                in_=bass.AP(tensor=wqkvT, offset=a * 128 * 384,
                            ap=[[384, 128], [128 * 384, b - a], [1, 384]]))
        elif pl == 3:
            # off the SP queue: small constants via the scalar engine
            for j in range(HL):
                nc.scalar.dma_start(out=k.qTa[j][64:65, :],
                                    in_=qrow[j:j + 1, :])
            nc.scalar.dma_start(out=k.btbl_t, in_=btbl[:, :])
            nc.gpsimd.memset(k.kTa[64:65, :], 1.0)
            nc.gpsimd.memset(k.v_aug[:, :, D:D + 1], 1.0)
            make_identity(nc, k.ident)
        elif pl == 4:  # Wo: first needed by outproj(0) units mid-phase 1
            nc.sync.dma_start(
                out=k.wo_res,
                in_=bass.AP(tensor=woT, offset=0,
                            ap=[[E, 128], [128 * E, 2], [1, E]]))
    k.wdma = wdma

    # bootstrap: phase 0 kv + group 0 emitted straight (DMA-paced); group 1
    # becomes dense filler so attention on slot pair (0,1) starts early
    _xt_dma(k, 0)
    steps0 = _proj_steps(k, 0)
    for step in steps0[:12]:       # kv chain+copy, transposes, g0 chain+copy
        step()
    k.dense.extend(("proj", 0, s) for s in steps0[12:])
    for q in range(NTQ):
        if q < NTQ - 1:
            _xt_dma(k, q + 1)
            ps = [("proj", q + 1, s) for s in _proj_steps(k, q + 1)]
            ops = list(k.dense)            # outproj units of q-1
            k.dense.clear()
            while ps or ops:               # round-robin merge
                if ops:
                    k.dense.append(ops.pop(0))
                if ps:
                    k.dense.append(ps.pop(0))
        _attn(k, q)
        k.drain_dense()            # any proj steps attn didn't absorb
        if q < NTQ - 1:
            _push_outproj(k, q)
    _tail_outproj(k)


def _xt_dma(k, q):
    """x column-slice load for phase q; first-e chunks split for fast start."""
    nc = k.nc
    cs = q * TQ
    xt = k.xtp.tile([128, NE, TQ], BF16, tag="xt", name=f"xt{q}")
    k.xt_cur = xt
    if q == 0:
        k.wdma(0)
        nc.sync.dma_start(
            out=xt[:, 0:1, :],
            in_=bass.AP(tensor=k.xT, offset=cs, ap=[[T, 128], [1, TQ]]))
        for pl, (a, b) in ((1, (1, 4)), (2, (4, 8)), (5, (8, 12)),
                           (6, (12, 16))):
            k.wdma(pl)
            nc.sync.dma_start(
                out=xt[:, a:b, :],
                in_=bass.AP(tensor=k.xT, offset=a * 128 * T + cs,
                            ap=[[T, 128], [128 * T, b - a], [1, TQ]]))
        k.wdma(3)
    else:
        nsp = 4 if q == 1 else 2
        for pl in range(nsp):
            w = NE // nsp
            nc.sync.dma_start(
                out=xt[:, w * pl:w * pl + w, :],
                in_=bass.AP(tensor=k.xT, offset=(w * pl * 128) * T + cs,
                            ap=[[T, 128], [128 * T, w], [1, TQ]]))
            if q == 1 and pl == 0:
                k.wdma(4)


def _proj_steps(k, q):
    """Projection for phase q as a list of dense-work closures: matmul
    bundles of 4 e-chunks, copy steps, and the v transposes."""
    nc = k.nc
    cs = q * TQ
    xt = k.xt_cur
    steps = []
    state = {}

    def chain_step(grp, e0):        # 4 accumulation matmuls
        def run():
            if e0 == 0:
                c0, c1 = grp * 128, (grp + 1) * 128
                if grp == 2:
                    c0, c1 = ES, ES + 2 * D
                state[grp] = (k.pup.tile([128, TQ], F32, tag="ps",
                                         name=f"acc{q}_{grp}"), c0, c1)
            acc, c0, c1 = state[grp]
            for e in range(e0, e0 + 4):
                nc.tensor.matmul(acc, k.wqkv_res[:, e, c0:c1], xt[:, e, :],
                                 start=(e == 0), stop=(e == NE - 1))
        return run

    def qcopy(grp):                 # split PSUM drain: ACT low / DVE shifted
        def run():
            acc = state[grp][0]
            nc.scalar.copy(out=k.qTa[2 * grp][0:64, cs:cs + TQ],
                           in_=acc[0:64, :])
            nc.vector.tensor_copy(out=k.qTa[2 * grp + 1][0:64, cs:cs + TQ],
                                  in_=acc[64:128, :])
        return run

    def kvcopy():
        def run():
            acc = state[2][0]
            nc.vector.tensor_copy(out=k.kTa[0:64, cs:cs + TQ],
                                  in_=acc[0:64, :])
            stv = k.stg.tile([128, TQ], BF16, tag="stg", name=f"stv{q}")
            nc.scalar.copy(out=stv[64:128, :], in_=acc[64:128, :])
            state["stv"] = stv
        return run

    def vtrans(mm):                 # v transpose via PE: [64,128] -> [128,64]
        def run():
            stv = state["stv"]
            tr = k.stp.tile([128, TQ], BF16, tag="st", name=f"tr{q}_{mm}")
            nc.tensor.transpose(tr[:, 0:D],
                                stv[64:128, mm * 128:(mm + 1) * 128],
                                k.ident[64:128, 64:128])
            nc.vector.tensor_copy(out=k.v_aug[:, 4 * q + mm, 0:D],
                                  in_=tr[:, 0:D])
        return run

    # kv first: phase 0 runs [kv, transposes, grp0] inline so attention on
    # slot pair (0,1) can start while grp1 is still DMA-paced.
    for grp in (2, 0, 1):
        for e0 in range(0, NE, 4):
            steps.append(chain_step(grp, e0))
        steps.append(qcopy(grp) if grp < 2 else kvcopy())
        if grp == 2:
            for mm in range(0, 4, 2):
                steps.append(lambda mm=mm: (vtrans(mm)(), vtrans(mm + 1)()))
    return steps


def _score_half(k, q, slot, rl, ti):
    """Score matmul + exp + causal mask for one tile; returns the g tile."""
    nc = k.nc
    cs = q * TQ
    tk, lo, hi = rl[ti]
    st = k.stp.tile([128, TQ], F32, tag="st")
    nc.tensor.matmul(st[:, lo - cs:hi - cs],
                     k.kTa[:, tk * 128:(tk + 1) * 128],
                     k.qTa[slot][:, lo:hi], start=True, stop=True)
    g = k.gp.tile([128, TQ], BF16, tag="g")
    if ti == 0 and hi - lo < TQ:
        nc.gpsimd.memset(g, 0.0)
    nc.scalar.activation(
        out=g[:, lo - cs:hi - cs], in_=st[:, lo - cs:hi - cs], func=EXP,
        bias=k.btbl_t[:, slot * NT128 + tk:slot * NT128 + tk + 1], scale=1.0)
    if tk >= 4 * q:  # diagonal block: causal mask, keep j <= i
        d0 = tk * 128 - cs
        nc.gpsimd.affine_select(
            out=g[:, d0:d0 + 128], in_=g[:, d0:d0 + 128],
            compare_op=mybir.AluOpType.is_ge,
            fill=0.0, base=0, pattern=[[1, 128]], channel_multiplier=-1)
    return g


def _av_half(k, q, slot, rl, ti, ot, g):
    """Accumulate one tile's g @ v into the head-output PSUM."""
    nc = k.nc
    cs = q * TQ
    tk, lo, hi = rl[ti]
    a_lo, a_hi = (0, TQ) if ti == 0 else (lo - cs, hi - cs)
    nc.tensor.matmul(ot[:, a_lo:a_hi], k.v_aug[:, tk, :], g[:, a_lo:a_hi],
                     start=(ti == 0), stop=(ti == len(rl) - 1))


def _norm(k, q, slot, ot):
    """headout = ot[0:64] / ot[64]; write into otn pair layout. The
    denominator row sits at PSUM partition 64; DVE handles the shifted
    reciprocal to partition 0 and the shifted odd-half multiply directly
    (verified on HW), so no staging DMAs are needed."""
    nc = k.nc
    cs, ce = q * TQ, (q + 1) * TQ
    rc = k.rcp.tile([1, TQ], F32, tag="rc")
    nc.vector.reciprocal(out=rc[0:1, :], in_=ot[64:65, :])
    bc = k.bcp.tile([64, TQ], F32, tag="bc")
    nc.gpsimd.partition_broadcast(bc, rc[0:1, :])
    pair, half = slot // 2, slot % 2
    nc.vector.tensor_mul(out=k.otn[pair][half * 64:half * 64 + 64, cs:ce],
                         in0=ot[0:64, :], in1=bc)


def _attn(k, q):
    """Attention for all 4 slots, pairwise interleaved, beat-scheduled:
    each beat emits one score tile, pops due AVs (SKEW behind), and pops
    dense work at a rate that exhausts the dense queue with the tiles."""
    seq = []
    for sA, sB in ((0, 1), (2, 3)):
        rlA, rlB = _tk_ranges(k_q := q, WS[sA]), _tk_ranges(q, WS[sB])
        otA = k.otp.tile([65, TQ], F32, tag="ot", name=f"ot{q}_{sA}")
        otB = k.otp.tile([65, TQ], F32, tag="ot", name=f"ot{q}_{sB}")
        for i in range(max(len(rlA), len(rlB))):
            if i < len(rlA):
                seq.append((sA, rlA, i, otA))
            if i < len(rlB):
                seq.append((sB, rlB, i, otB))
    for n, (slot, rl, i, ot) in enumerate(seq):
        g = _score_half(k, q, slot, rl, i)
        k.avq.append((slot, rl, i, ot, g, q))
        if len(k.avq) > SKEW:
            k.pop_av()
        left = len(seq) - n - 1
        ndense = len(k.dense) if left == 0 else (len(k.dense) + left - 1) // left
        for _ in range(min(ndense, 3 if left else len(k.dense))):
            if k.dense:
                k.pop_dense()


def _tail_outproj(k):
    """Last phase's output projection. otn[0] (slot pair 0,1) is final
    before the last AV drain, so those half-matmuls preheat PSUM banks while
    the drain's norm chains run; otn[1] halves, copies, and split DMAs
    follow. Keeps the PE fed through the very end."""
    nc = k.nc
    t0 = 4 * (NTQ - 1)
    obs = {}

    def ensure_ob(t):
        if t not in obs:
            obs[t] = k.osp.tile([128, 4, TQ], BF16, tag="ob", name=f"tob{t}")
        return obs[t]

    def a_half(t, o):
        pool, tag = (k.pup, "ps") if (t + o) % 2 == 0 else (k.stp, "st")
        po = pool.tile([128, TQ], F32, tag=tag, name=f"tpo{t}_{o}")
        nc.tensor.matmul(po, k.otn[0][:, t * 128:(t + 1) * 128],
                         k.wo_res[:, 0, o * TQ:(o + 1) * TQ],
                         start=True, stop=False)
        return po

    def finish(t, o, po):
        nc.tensor.matmul(po, k.otn[1][:, t * 128:(t + 1) * 128],
                         k.wo_res[:, 1, o * TQ:(o + 1) * TQ],
                         start=False, stop=True)
        ob = ensure_ob(t)
        if o % 2:
            nc.scalar.copy(out=ob[:, o, :], in_=po)
        else:
            nc.vector.tensor_copy(out=ob[:, o, :], in_=po)
        if o == 1:
            nc.sync.dma_start(out=k.part[t * 128:(t + 1) * 128, 0:2 * TQ],
                              in_=ob[:, 0:2, :])
        elif o == 3:
            nc.sync.dma_start(out=k.part[t * 128:(t + 1) * 128, 2 * TQ:4 * TQ],
                              in_=ob[:, 2:4, :])

    pre = [(t0, 0), (t0, 1), (t0 + 1, 0), (t0 + 1, 1), (t0 + 2, 0), (t0 + 2, 1)]
    pos = {}
    for t, o in pre:
        pos[(t, o)] = a_half(t, o)
        for _ in range(2):
            if k.avq:
                k.pop_av()
    k.drain_av()
    for t, o in pre:
        finish(t, o, pos[(t, o)])
    rest = [(t0 + 3, 0), (t0 + 3, 1), (t0, 2), (t0, 3), (t0 + 1, 2),
            (t0 + 1, 3), (t0 + 2, 2), (t0 + 2, 3), (t0 + 3, 2), (t0 + 3, 3)]
    for t, o in rest:
        finish(t, o, a_half(t, o))


def _push_outproj(k, q, tail=False):
    """Queue output projection for phase q's 4 token blocks as dense units.
    In the tail (no attention left to hide behind), po tiles alternate
    between the two PSUM pools so copies never gate the next matmul."""
    nc = k.nc
    state = {}

    def unit(t, o, use_st):
        def run():
            if o == 0:
                state[t] = k.osp.tile([128, 4, TQ], BF16, tag="ob",
                                      name=f"ob{t}")
            ob = state[t]
            pool, tag = (k.stp, "st") if use_st else (k.pup, "ps")
            po = pool.tile([128, TQ], F32, tag=tag, name=f"po{t}_{o}")
            nc.tensor.matmul(po, k.otn[0][:, t * 128:(t + 1) * 128],
                             k.wo_res[:, 0, o * TQ:(o + 1) * TQ],
                             start=True, stop=False)
            nc.tensor.matmul(po, k.otn[1][:, t * 128:(t + 1) * 128],
                             k.wo_res[:, 1, o * TQ:(o + 1) * TQ],
                             start=False, stop=True)
            if o % 2:
                nc.scalar.copy(out=ob[:, o, :], in_=po)
            else:
                nc.vector.tensor_copy(out=ob[:, o, :], in_=po)
            if tail:  # halve drain latency: ship each ob half as it fills
                if o == 1:
                    nc.sync.dma_start(
                        out=k.part[t * 128:(t + 1) * 128, 0:2 * TQ],
                        in_=ob[:, 0:2, :])
                elif o == 3:
                    nc.sync.dma_start(
                        out=k.part[t * 128:(t + 1) * 128, 2 * TQ:4 * TQ],
                        in_=ob[:, 2:4, :])
            elif o == 3:
                nc.sync.dma_start(out=k.part[t * 128:(t + 1) * 128, :],
                                  in_=ob[:, :, :])
        return run

    n = 0
    for t in range(4 * q, 4 * q + 4):
        for o in range(4):
            k.dense.append(("op", q, unit(t, o, tail and (n % 2 == 1))))
            n += 1


def _prepare_in_maps(x, Wq, Wk, Wv, Wo):
    xTn = np.ascontiguousarray(x[0].T).astype(NPBF16)
    scale = np.float64(D) ** -0.5
    i = np.arange(T, dtype=np.float64)
    p = np.arange(128, dtype=np.float64)
    kk = np.arange(NT128, dtype=np.float64)
    in_maps = []
    for c in range(NCORES):
        hs = [24 + c, c, 16 + c, 8 + c]   # window profile WS = [16, 1, 10, 3]
        wq_rows = np.concatenate(
            [Wq[h * D:(h + 1) * D, :] * scale for h in hs], axis=0)  # [256, E]
        wkv = np.concatenate([Wk, Wv], axis=0)                       # [128, E]
        wqkvT = np.ascontiguousarray(
            np.concatenate([wq_rows, wkv], axis=0).T).astype(NPBF16)
        woT = np.ascontiguousarray(
            np.concatenate([Wo[:, h * D:(h + 1) * D] for h in hs], axis=1).T
        ).astype(NPBF16)
        slopes = np.power(2.0, -8.0 * (np.asarray(hs, np.float64) + 1.0) / H)
        qrow_n = (-slopes[:, None] * i[None, :]).astype(NPBF16)      # [HL, T]
        btbl_n = (slopes[:, None, None] * (kk[None, :, None] * 128 + p[None, None, :]))
        btbl_n = np.ascontiguousarray(
            btbl_n.transpose(2, 0, 1).reshape(128, HL * NT128)).astype(np.float32)
        in_maps.append({
            "xT": xTn, "wqkvT": wqkvT, "woT": woT,
            "qrow": qrow_n, "btbl": btbl_n,
        })
    return in_maps


def kernel(x, Wq, Wk, Wv, Wo, attention_mask, _trace=False, _trace_cores=None):
    x = np.asarray(x, dtype=np.float32)
    Wq = np.asarray(Wq, dtype=np.float32)
    Wk = np.asarray(Wk, dtype=np.float32)
    Wv = np.asarray(Wv, dtype=np.float32)
    Wo = np.asarray(Wo, dtype=np.float32)

    if "nc" not in _CACHE:
        _CACHE["nc"] = _build_nc()
    nc = _CACHE["nc"]

    in_maps = _prepare_in_maps(x, Wq, Wk, Wv, Wo)
    kwargs = {}
    if _trace:
        kwargs = {"trace": True, "trace_cores": _trace_cores or [0]}
    res = run_bass_kernel_spmd(nc, in_maps, core_ids=list(range(NCORES)), **kwargs)
    acc = np.zeros((T, E), dtype=np.float64)
    for r in res.results:
        acc += np.asarray(r["part"]).astype(np.float64)
    out = acc.astype(np.float32)[None, :, :]
    if _trace:
        _CACHE["last_result"] = res
    return out


# revision 6
# speedup vs baseline: 1.0365x; 1.0029x over previous
"""MQA attention (32 query heads, 1 KV head, ALiBi, causal) on 8 trn2 cores.

Sharding: tensor-parallel over query heads (4 heads/core). Wq rows and Wo
columns are sharded; x, Wk, Wv are replicated. Each core computes a partial
[T, E] output (its 4 heads pushed through its Wo column-shard); the host sums
the 8 partials.

v2 design vs baseline:
- bf16 operands everywhere on the PE (1 cyc/col at any width, no fp32r
  256-col padding); fp32 accumulation in PSUM; partial output written bf16.
- ALiBi-windowed attention: head h's weights decay as exp(-s_h * dist), so
  keys beyond dist tau/s_h contribute < e^-20 relative and are skipped at
  128-block granularity. Cores get head sets {24+c, 16+c, 8+c, c} with
  identical window profiles W = [16(full), 10, 3, 1] blocks, so the SPMD
  instruction stream is core-independent and load-balanced.
- Few large DMAs (HWDGE serializes at ~625ns/DMA): x loaded in big strided
  DMAs, weights in 4, output staged to [128, 2048] bf16 rows.
- The PE executes strictly in program order, so emission order IS the
  schedule: score->AV skew of 4 tiles, AV/outproj work deferred across
  phase boundaries as filler between other PE ops, so the PE never sits
  behind a parked instruction waiting on the ACT/Pool exp/mask chain.

Math per core c (slots j=0..3, heads hs=[24+c, 16+c, 8+c, c]):
  qT_j = (Wq_hj * D^-0.5) @ x^T                    [64, T]
  kT   = Wk @ x^T, v = (Wv @ x^T)^T                [64, T], [T, 64]
  ST[j, i] = kT[:,j].q  +  (-s_h * i)              via augmented row (ones in
                                                   kTa row 64, -s_h*i in qTa)
  g = exp(ST + s_h*j)      (per-partition ACT bias; per-column factors cancel
                            in softmax normalization; causal mask via
                            affine_select on diagonal blocks; key blocks
                            outside the ALiBi window skipped)
  OT_aug = [v | 1]^T @ g   -> rows 0:64 = unnormalized head out^T,
                              row 64    = softmax denominator
  headout^T = OT / denom   (partition_broadcast of 1/denom)
  partial = headout^T.T @ WoT_shard                [T, E] bf16
"""

from collections import deque

import numpy as np
import ml_dtypes

import concourse.bacc as bacc
import concourse.bass as bass
import concourse.mybir as mybir
import concourse.tile as tile
from concourse.masks import make_identity
from concourse.bass_utils import run_bass_kernel_spmd

T = 2048          # tokens
E = 2048          # embed dim
H = 32            # query heads
D = 64            # head dim
NCORES = 8
HL = H // NCORES  # 4 heads per core
ES = HL * D       # 256 = per-core E shard
TQ = 512          # query-phase tile
NTQ = T // TQ     # 4
NE = E // 128     # 16 contraction chunks
NT128 = T // 128  # 16

WS = [16, 1, 10, 3]   # per-slot ALiBi windows (key blocks beyond diagonal);
                      # pairs (0,1) and (2,3) balance long+short tile lists
SKEW = 5              # score -> AV deferral depth (tiles)

F32 = mybir.dt.float32
BF16 = mybir.dt.bfloat16
EXP = mybir.ActivationFunctionType.Exp
NPBF16 = ml_dtypes.bfloat16

_CACHE = {}


def _tk_ranges(q, W):
    """(tk, lo, hi) global-column score tiles for query block q, window W."""
    cs, ce = q * TQ, (q + 1) * TQ
    out = []
    for tk in range(max(0, 4 * q - W), 4 * q + 4):
        lo = max(cs, tk * 128)
        hi = min(ce, (tk + W + 1) * 128)
        out.append((tk, lo, hi))
    return out


def _build_nc():
    nc = bacc.Bacc("TRN2")
    xT = nc.dram_tensor("xT", [E, T], BF16, kind="ExternalInput")
    wqkvT = nc.dram_tensor("wqkvT", [E, ES + 2 * D], BF16, kind="ExternalInput")
    woT = nc.dram_tensor("woT", [ES, E], BF16, kind="ExternalInput")
    qrow = nc.dram_tensor("qrow", [HL, T], BF16, kind="ExternalInput")
    btbl = nc.dram_tensor("btbl", [128, HL * NT128], F32, kind="ExternalInput")
    part = nc.dram_tensor("part", [T, E], BF16, kind="ExternalOutput")

    from contextlib import ExitStack
    with tile.TileContext(nc) as tc, ExitStack() as ctx:
        _body(nc, tc, ctx, xT, wqkvT, woT, qrow, btbl, part)
    nc.finalize()
    return nc


class _K:
    """Kernel emission state: tile pools, resident tiles, and the deferral
    queues. The PE runs strictly in program order, so emission order is the
    schedule: attention tiles (whose g comes back through the ACT/Pool
    exp/mask chain) are interleaved beat-by-beat with "dense" PE work
    (projection chains, output projection) that has no cross-engine latency.
    AV matmuls pop SKEW tiles after their score."""

    def pop_av(self):
        slot, rl, i, ot, g, q = self.avq.popleft()
        _av_half(self, q, slot, rl, i, ot, g)
        if i == len(rl) - 1:
            _norm(self, q, slot, ot)

    def pop_dense(self):
        """Run one dense unit. An outproj unit of phase q reads otn columns
        written by phase q's norms, so every pending AV of phase <= q must be
        emitted first (emission order IS dependency order for the tile
        framework: a read emitted before its writer reads stale data)."""
        kind, qu, run = self.dense[0]
        if kind == "op" and self.avq and self.avq[0][5] <= qu:
            self.pop_av()
            return
        self.dense.popleft()
        run()

    def drain_av(self):
        while self.avq:
            self.pop_av()

    def drain_dense(self):
        while self.dense:
            self.pop_dense()


def _body(nc, tc, ctx, xT, wqkvT, woT, qrow, btbl, part):
    k = _K()
    k.nc = nc
    k.part = part
    k.xT = xT
    k.avq = deque()
    k.dense = deque()

    const = ctx.enter_context(tc.tile_pool(name="const", bufs=1))
    k.xtp = ctx.enter_context(tc.tile_pool(name="xt", bufs=2))
    k.stg = ctx.enter_context(tc.tile_pool(name="stg", bufs=3))
    k.gp = ctx.enter_context(tc.tile_pool(name="g", bufs=8))
    k.rcp = ctx.enter_context(tc.tile_pool(name="rc", bufs=3))
    k.bcp = ctx.enter_context(tc.tile_pool(name="bc", bufs=3))
    k.osp = ctx.enter_context(tc.tile_pool(name="ostage", bufs=4))

    # ---------- resident constants ----------------------------------------
    k.wqkv_res = const.tile([128, NE, ES + 2 * D], BF16)
    k.wo_res = const.tile([128, 2, E], BF16)
    k.qTa = []
    for j in range(HL):
        qa = const.tile([65, T], BF16, tag=f"qTa{j}")
        k.qTa.append(qa)
    k.kTa = const.tile([65, T], BF16)
    k.v_aug = const.tile([128, NT128, D + 1], BF16)
    k.btbl_t = const.tile([128, HL * NT128], F32)
    k.ident = const.tile([128, 128], BF16)
    k.otn = []
    for p2 in range(2):
        o = const.tile([128, T], BF16, tag=f"otn{p2}")
        k.otn.append(o)

    # ---------- 8 PSUM banks: (acc|po) 2 + st 4 + ot 2 --------------------
    k.pup = ctx.enter_context(tc.tile_pool(name="ps_acc", bufs=2, space="PSUM"))
    k.stp = ctx.enter_context(tc.tile_pool(name="st_ps", bufs=4, space="PSUM"))
    k.otp = ctx.enter_context(tc.tile_pool(name="ot_ps", bufs=2, space="PSUM"))

    def wdma(pl):  # weight loads, interleaved chunk-by-chunk with x at q0
        if pl == 0:
            nc.sync.dma_start(
                out=k.wqkv_res[:, 0:1, :],
                in_=bass.AP(tensor=wqkvT, offset=0,
                            ap=[[384, 128], [1, 384]]))
        elif pl in (1, 2, 5, 6):
            a, b = {1: (1, 4), 2: (4, 8), 5: (8, 12), 6: (12, 16)}[pl]
            nc.scalar.dma_start(
                out=k.wqkv_res[:, a:b, :],
                in_=bass.AP(tensor=wqkvT, offset=a * 128 * 384,
                            ap=[[384, 128], [128 * 384, b - a], [1, 384]]))
        elif pl == 3:
            # off the SP queue: small constants via the scalar engine
            for j in range(HL):
                nc.scalar.dma_start(out=k.qTa[j][64:65, :],
                                    in_=qrow[j:j + 1, :])
            nc.scalar.dma_start(out=k.btbl_t, in_=btbl[:, :])
            nc.gpsimd.memset(k.kTa[64:65, :], 1.0)
            nc.gpsimd.memset(k.v_aug[:, :, D:D + 1], 1.0)
            make_identity(nc, k.ident)
        elif pl == 4:  # Wo: first needed by outproj(0) units mid-phase 1
            nc.sync.dma_start(
                out=k.wo_res,
                in_=bass.AP(tensor=woT, offset=0,
                            ap=[[E, 128], [128 * E, 2], [1, E]]))
    k.wdma = wdma

    # bootstrap: phase 0 kv + group 0 emitted straight (DMA-paced); group 1
    # becomes dense filler so attention on slot pair (0,1) starts early
    _xt_dma(k, 0)
    steps0 = _proj_steps(k, 0)
    for step in steps0[:12]:       # kv chain+copy, transposes, g0 chain+copy
        step()
    k.dense.extend(("proj", 0, s) for s in steps0[12:])
    for q in range(NTQ):
        if q < NTQ - 1:
            _xt_dma(k, q + 1)
            ps = [("proj", q + 1, s) for s in _proj_steps(k, q + 1)]
            ops = list(k.dense)            # outproj units of q-1
            k.dense.clear()
            while ps or ops:               # round-robin merge
                if ops:
                    k.dense.append(ops.pop(0))
                if ps:
                    k.dense.append(ps.pop(0))
        _attn(k, q)
        k.drain_dense()            # any proj steps attn didn't absorb
        if q < NTQ - 1:
            _push_outproj(k, q, all_dve=(q == NTQ - 2))
    _tail_outproj(k)


def _xt_dma(k, q):
    """x column-slice load for phase q; first-e chunks split for fast start."""
    nc = k.nc
    cs = q * TQ
    xt = k.xtp.tile([128, NE, TQ], BF16, tag="xt", name=f"xt{q}")
    k.xt_cur = xt
    if q == 0:
        k.wdma(0)
        nc.sync.dma_start(
            out=xt[:, 0:1, :],
            in_=bass.AP(tensor=k.xT, offset=cs, ap=[[T, 128], [1, TQ]]))
        for pl, (a, b) in ((1, (1, 4)), (2, (4, 8)), (5, (8, 12)),
                           (6, (12, 16))):
            k.wdma(pl)
            nc.sync.dma_start(
                out=xt[:, a:b, :],
                in_=bass.AP(tensor=k.xT, offset=a * 128 * T + cs,
                            ap=[[T, 128], [128 * T, b - a], [1, TQ]]))
        k.wdma(3)
    else:
        nsp = 4 if q == 1 else 2
        for pl in range(nsp):
            w = NE // nsp
            nc.sync.dma_start(
                out=xt[:, w * pl:w * pl + w, :],
                in_=bass.AP(tensor=k.xT, offset=(w * pl * 128) * T + cs,
                            ap=[[T, 128], [128 * T, w], [1, TQ]]))
            if q == 1 and pl == 0:
                k.wdma(4)


def _proj_steps(k, q):
    """Projection for phase q as a list of dense-work closures: matmul
    bundles of 4 e-chunks, copy steps, and the v transposes."""
    nc = k.nc
    cs = q * TQ
    xt = k.xt_cur
    steps = []
    state = {}

    def chain_step(grp, e0):        # 4 accumulation matmuls
        def run():
            if e0 == 0:
                c0, c1 = grp * 128, (grp + 1) * 128
                if grp == 2:
                    c0, c1 = ES, ES + 2 * D
                state[grp] = (k.pup.tile([128, TQ], F32, tag="ps",
                                         name=f"acc{q}_{grp}"), c0, c1)
            acc, c0, c1 = state[grp]
            for e in range(e0, e0 + 4):
                nc.tensor.matmul(acc, k.wqkv_res[:, e, c0:c1], xt[:, e, :],
                                 start=(e == 0), stop=(e == NE - 1))
        return run

    def qcopy(grp):                 # split PSUM drain: ACT low / DVE shifted
        def run():
            acc = state[grp][0]
            nc.scalar.copy(out=k.qTa[2 * grp][0:64, cs:cs + TQ],
                           in_=acc[0:64, :])
            nc.vector.tensor_copy(out=k.qTa[2 * grp + 1][0:64, cs:cs + TQ],
                                  in_=acc[64:128, :])
        return run

    def kvcopy():
        def run():
            acc = state[2][0]
            nc.vector.tensor_copy(out=k.kTa[0:64, cs:cs + TQ],
                                  in_=acc[0:64, :])
            stv = k.stg.tile([128, TQ], BF16, tag="stg", name=f"stv{q}")
            nc.scalar.copy(out=stv[64:128, :], in_=acc[64:128, :])
            state["stv"] = stv
        return run

    def vtrans(mm):                 # v transpose via PE: [64,128] -> [128,64]
        def run():
            stv = state["stv"]
            tr = k.stp.tile([128, TQ], BF16, tag="st", name=f"tr{q}_{mm}")
            nc.tensor.transpose(tr[:, 0:D],
                                stv[64:128, mm * 128:(mm + 1) * 128],
                                k.ident[64:128, 64:128])
            nc.vector.tensor_copy(out=k.v_aug[:, 4 * q + mm, 0:D],
                                  in_=tr[:, 0:D])
        return run

    # kv first: phase 0 runs [kv, transposes, grp0] inline so attention on
    # slot pair (0,1) can start while grp1 is still DMA-paced.
    for grp in (2, 0, 1):
        for e0 in range(0, NE, 4):
            steps.append(chain_step(grp, e0))
        steps.append(qcopy(grp) if grp < 2 else kvcopy())
        if grp == 2:
            for mm in range(0, 4, 2):
                steps.append(lambda mm=mm: (vtrans(mm)(), vtrans(mm + 1)()))
    return steps


def _score_half(k, q, slot, rl, ti):
    """Score matmul + exp + causal mask for one tile; returns the g tile."""
    nc = k.nc
    cs = q * TQ
    tk, lo, hi = rl[ti]
    st = k.stp.tile([128, TQ], F32, tag="st")
    nc.tensor.matmul(st[:, lo - cs:hi - cs],
                     k.kTa[:, tk * 128:(tk + 1) * 128],
                     k.qTa[slot][:, lo:hi], start=True, stop=True)
    g = k.gp.tile([128, TQ], BF16, tag="g")
    if ti == 0 and hi - lo < TQ:
        nc.gpsimd.memset(g, 0.0)
    nc.scalar.activation(
        out=g[:, lo - cs:hi - cs], in_=st[:, lo - cs:hi - cs], func=EXP,
        bias=k.btbl_t[:, slot * NT128 + tk:slot * NT128 + tk + 1], scale=1.0)
    if tk >= 4 * q:  # diagonal block: causal mask, keep j <= i
        d0 = tk * 128 - cs
        nc.gpsimd.affine_select(
            out=g[:, d0:d0 + 128], in_=g[:, d0:d0 + 128],
            compare_op=mybir.AluOpType.is_ge,
            fill=0.0, base=0, pattern=[[1, 128]], channel_multiplier=-1)
    return g


def _av_half(k, q, slot, rl, ti, ot, g):
    """Accumulate one tile's g @ v into the head-output PSUM."""
    nc = k.nc
    cs = q * TQ
    tk, lo, hi = rl[ti]
    a_lo, a_hi = (0, TQ) if ti == 0 else (lo - cs, hi - cs)
    nc.tensor.matmul(ot[:, a_lo:a_hi], k.v_aug[:, tk, :], g[:, a_lo:a_hi],
                     start=(ti == 0), stop=(ti == len(rl) - 1))


def _norm(k, q, slot, ot):
    """headout = ot[0:64] / ot[64]; write into otn pair layout. The
    denominator row sits at PSUM partition 64; DVE handles the shifted
    reciprocal to partition 0 and the shifted odd-half multiply directly
    (verified on HW), so no staging DMAs are needed."""
    nc = k.nc
    cs, ce = q * TQ, (q + 1) * TQ
    rc = k.rcp.tile([1, TQ], F32, tag="rc")
    nc.vector.reciprocal(out=rc[0:1, :], in_=ot[64:65, :])
    bc = k.bcp.tile([64, TQ], F32, tag="bc")
    nc.gpsimd.partition_broadcast(bc, rc[0:1, :])
    pair, half = slot // 2, slot % 2
    nc.vector.tensor_mul(out=k.otn[pair][half * 64:half * 64 + 64, cs:ce],
                         in0=ot[0:64, :], in1=bc)


def _attn(k, q):
    """Attention for all 4 slots, pairwise interleaved, beat-scheduled:
    each beat emits one score tile, pops due AVs (SKEW behind), and pops
    dense work at a rate that exhausts the dense queue with the tiles."""
    seq = []
    for sA, sB in ((0, 1), (2, 3)):
        rlA, rlB = _tk_ranges(k_q := q, WS[sA]), _tk_ranges(q, WS[sB])
        otA = k.otp.tile([65, TQ], F32, tag="ot", name=f"ot{q}_{sA}")
        otB = k.otp.tile([65, TQ], F32, tag="ot", name=f"ot{q}_{sB}")
        for i in range(max(len(rlA), len(rlB))):
            if i < len(rlA):
                seq.append((sA, rlA, i, otA))
            if i < len(rlB):
                seq.append((sB, rlB, i, otB))
    for n, (slot, rl, i, ot) in enumerate(seq):
        g = _score_half(k, q, slot, rl, i)
        k.avq.append((slot, rl, i, ot, g, q))
        if len(k.avq) > SKEW:
            k.pop_av()
        left = len(seq) - n - 1
        ndense = len(k.dense) if left == 0 else (len(k.dense) + left - 1) // left
        for _ in range(min(ndense, 3 if left else len(k.dense))):
            if k.dense:
                k.pop_dense()


def _tail_outproj(k):
    """Last phase's output projection. otn[0] (slot pair 0,1) is final
    before the last AV drain, so those half-matmuls preheat PSUM banks while
    the drain's norm chains run; otn[1] halves, copies, and split DMAs
    follow. Keeps the PE fed through the very end."""
    nc = k.nc
    t0 = 4 * (NTQ - 1)
    obs = {}

    def ensure_ob(t):
        if t not in obs:
            obs[t] = k.osp.tile([128, 4, TQ], BF16, tag="ob", name=f"tob{t}")
        return obs[t]

    def a_half(t, o):
        pool, tag = (k.pup, "ps") if (t + o) % 2 == 0 else (k.stp, "st")
        po = pool.tile([128, TQ], F32, tag=tag, name=f"tpo{t}_{o}")
        nc.tensor.matmul(po, k.otn[0][:, t * 128:(t + 1) * 128],
                         k.wo_res[:, 0, o * TQ:(o + 1) * TQ],
                         start=True, stop=False)
        return po

    def finish(t, o, po):
        nc.tensor.matmul(po, k.otn[1][:, t * 128:(t + 1) * 128],
                         k.wo_res[:, 1, o * TQ:(o + 1) * TQ],
                         start=False, stop=True)
        ob = ensure_ob(t)
        if o % 2:
            nc.scalar.copy(out=ob[:, o, :], in_=po)
        else:
            nc.vector.tensor_copy(out=ob[:, o, :], in_=po)
        if o == 1:
            nc.sync.dma_start(out=k.part[t * 128:(t + 1) * 128, 0:2 * TQ],
                              in_=ob[:, 0:2, :])
        elif o == 3:
            nc.sync.dma_start(out=k.part[t * 128:(t + 1) * 128, 2 * TQ:4 * TQ],
                              in_=ob[:, 2:4, :])

    pre = [(t0, 0), (t0, 1), (t0 + 1, 0), (t0 + 1, 1), (t0 + 2, 0), (t0 + 2, 1)]
    pos = {}
    for t, o in pre:
        pos[(t, o)] = a_half(t, o)
        for _ in range(2):
            if k.avq:
                k.pop_av()
    k.drain_av()
    for t, o in pre:
        finish(t, o, pos[(t, o)])
    rest = [(t0 + 3, 0), (t0 + 3, 1), (t0, 2), (t0, 3), (t0 + 1, 2),
            (t0 + 1, 3), (t0 + 2, 2), (t0 + 2, 3), (t0 + 3, 2), (t0 + 3, 3)]
    for t, o in rest:
        finish(t, o, a_half(t, o))


def _push_outproj(k, q, all_dve=False):
    """Queue output projection for phase q's 4 token blocks as dense units.
    These pop as filler during phase q+1; for q = 2 (popping during the
    ACT-saturated phase 3) all copies go to DVE to keep exps flowing."""
    nc = k.nc
    state = {}

    def unit(t, o, use_st):
        def run():
            if o == 0:
                state[t] = k.osp.tile([128, 4, TQ], BF16, tag="ob",
                                      name=f"ob{t}")
            ob = state[t]
            pool, tag = (k.stp, "st") if use_st else (k.pup, "ps")
            po = pool.tile([128, TQ], F32, tag=tag, name=f"po{t}_{o}")
            nc.tensor.matmul(po, k.otn[0][:, t * 128:(t + 1) * 128],
                             k.wo_res[:, 0, o * TQ:(o + 1) * TQ],
                             start=True, stop=False)
            nc.tensor.matmul(po, k.otn[1][:, t * 128:(t + 1) * 128],
                             k.wo_res[:, 1, o * TQ:(o + 1) * TQ],
                             start=False, stop=True)
            if o % 2 and not all_dve:
                nc.scalar.copy(out=ob[:, o, :], in_=po)
            else:
                nc.vector.tensor_copy(out=ob[:, o, :], in_=po)
            if o == 3:
                nc.sync.dma_start(out=k.part[t * 128:(t + 1) * 128, :],
                                  in_=ob[:, :, :])
        return run

    for t in range(4 * q, 4 * q + 4):
        for o in range(4):
            k.dense.append(("op", q, unit(t, o, False)))


def _prepare_in_maps(x, Wq, Wk, Wv, Wo):
    xTn = np.ascontiguousarray(x[0].T).astype(NPBF16)
    scale = np.float64(D) ** -0.5
    i = np.arange(T, dtype=np.float64)
    p = np.arange(128, dtype=np.float64)
    kk = np.arange(NT128, dtype=np.float64)
    in_maps = []
    for c in range(NCORES):
        hs = [24 + c, c, 16 + c, 8 + c]   # window profile WS = [16, 1, 10, 3]
        wq_rows = np.concatenate(
            [Wq[h * D:(h + 1) * D, :] * scale for h in hs], axis=0)  # [256, E]
        wkv = np.concatenate([Wk, Wv], axis=0)                       # [128, E]
        wqkvT = np.ascontiguousarray(
            np.concatenate([wq_rows, wkv], axis=0).T).astype(NPBF16)
        woT = np.ascontiguousarray(
            np.concatenate([Wo[:, h * D:(h + 1) * D] for h in hs], axis=1).T
        ).astype(NPBF16)
        slopes = np.power(2.0, -8.0 * (np.asarray(hs, np.float64) + 1.0) / H)
        qrow_n = (-slopes[:, None] * i[None, :]).astype(NPBF16)      # [HL, T]
        btbl_n = (slopes[:, None, None] * (kk[None, :, None] * 128 + p[None, None, :]))
        btbl_n = np.ascontiguousarray(
            btbl_n.transpose(2, 0, 1).reshape(128, HL * NT128)).astype(np.float32)
        in_maps.append({
            "xT": xTn, "wqkvT": wqkvT, "woT": woT,
            "qrow": qrow_n, "btbl": btbl_n,
        })
    return in_maps


def kernel(x, Wq, Wk, Wv, Wo, attention_mask, _trace=False, _trace_cores=None):
    x = np.asarray(x, dtype=np.float32)
    Wq = np.asarray(Wq, dtype=np.float32)
    Wk = np.asarray(Wk, dtype=np.float32)
    Wv = np.asarray(Wv, dtype=np.float32)
    Wo = np.asarray(Wo, dtype=np.float32)

    if "nc" not in _CACHE:
        _CACHE["nc"] = _build_nc()
    nc = _CACHE["nc"]

    in_maps = _prepare_in_maps(x, Wq, Wk, Wv, Wo)
    kwargs = {}
    if _trace:
        kwargs = {"trace": True, "trace_cores": _trace_cores or [0]}
    res = run_bass_kernel_spmd(nc, in_maps, core_ids=list(range(NCORES)), **kwargs)
    acc = np.zeros((T, E), dtype=np.float64)
    for r in res.results:
        acc += np.asarray(r["part"]).astype(np.float64)
    out = acc.astype(np.float32)[None, :, :]
    if _trace:
        _CACHE["last_result"] = res
    return out


# revision 7
# speedup vs baseline: 1.0567x; 1.0194x over previous
"""MQA attention (32 query heads, 1 KV head, ALiBi, causal) on 8 trn2 cores.

Sharding: tensor-parallel over query heads (4 heads/core). Wq rows and Wo
columns are sharded; x, Wk, Wv are replicated. Each core computes a partial
[T, E] output (its 4 heads pushed through its Wo column-shard); the host sums
the 8 partials.

v2 design vs baseline:
- bf16 operands everywhere on the PE (1 cyc/col at any width, no fp32r
  256-col padding); fp32 accumulation in PSUM; partial output written bf16.
- ALiBi-windowed attention: head h's weights decay as exp(-s_h * dist), so
  keys beyond dist tau/s_h contribute < e^-20 relative and are skipped at
  128-block granularity. Cores get head sets {24+c, 16+c, 8+c, c} with
  identical window profiles W = [16(full), 10, 3, 1] blocks, so the SPMD
  instruction stream is core-independent and load-balanced.
- Few large DMAs (HWDGE serializes at ~625ns/DMA): x loaded in big strided
  DMAs, weights in 4, output staged to [128, 2048] bf16 rows.
- The PE executes strictly in program order, so emission order IS the
  schedule: score->AV skew of 4 tiles, AV/outproj work deferred across
  phase boundaries as filler between other PE ops, so the PE never sits
  behind a parked instruction waiting on the ACT/Pool exp/mask chain.

Math per core c (slots j=0..3, heads hs=[24+c, 16+c, 8+c, c]):
  qT_j = (Wq_hj * D^-0.5) @ x^T                    [64, T]
  kT   = Wk @ x^T, v = (Wv @ x^T)^T                [64, T], [T, 64]
  ST[j, i] = kT[:,j].q  +  (-s_h * i)              via augmented row (ones in
                                                   kTa row 64, -s_h*i in qTa)
  g = exp(ST + s_h*j)      (per-partition ACT bias; per-column factors cancel
                            in softmax normalization; causal mask via
                            affine_select on diagonal blocks; key blocks
                            outside the ALiBi window skipped)
  OT_aug = [v | 1]^T @ g   -> rows 0:64 = unnormalized head out^T,
                              row 64    = softmax denominator
  headout^T = OT / denom   (partition_broadcast of 1/denom)
  partial = headout^T.T @ WoT_shard                [T, E] bf16
"""

from collections import deque

import numpy as np
import ml_dtypes

import concourse.bacc as bacc
import concourse.bass as bass
import concourse.mybir as mybir
import concourse.tile as tile
from concourse.masks import make_identity
from concourse.bass_utils import run_bass_kernel_spmd

T = 2048          # tokens
E = 2048          # embed dim
H = 32            # query heads
D = 64            # head dim
NCORES = 8
HL = H // NCORES  # 4 heads per core
ES = HL * D       # 256 = per-core E shard
TQ = 512          # query-phase tile
NTQ = T // TQ     # 4
NE = E // 128     # 16 contraction chunks
NT128 = T // 128  # 16

WS = [16, 1, 10, 3]   # per-slot ALiBi windows (key blocks beyond diagonal);
                      # pairs (0,1) and (2,3) balance long+short tile lists
SKEW = 5              # score -> AV deferral depth (tiles)

F32 = mybir.dt.float32
BF16 = mybir.dt.bfloat16
EXP = mybir.ActivationFunctionType.Exp
NPBF16 = ml_dtypes.bfloat16

_CACHE = {}


def _tk_ranges(q, W):
    """(tk, lo, hi) global-column score tiles for query block q, window W."""
    cs, ce = q * TQ, (q + 1) * TQ
    out = []
    for tk in range(max(0, 4 * q - W), 4 * q + 4):
        lo = max(cs, tk * 128)
        hi = min(ce, (tk + W + 1) * 128)
        out.append((tk, lo, hi))
    return out


def _build_nc():
    nc = bacc.Bacc("TRN2")
    xT = nc.dram_tensor("xT", [E, T], BF16, kind="ExternalInput")
    wqkvT = nc.dram_tensor("wqkvT", [E, ES + 2 * D], BF16, kind="ExternalInput")
    woT = nc.dram_tensor("woT", [ES, E], BF16, kind="ExternalInput")
    qrow = nc.dram_tensor("qrow", [HL, T], BF16, kind="ExternalInput")
    btbl = nc.dram_tensor("btbl", [128, HL * NT128], F32, kind="ExternalInput")
    part = nc.dram_tensor("part", [T, E], BF16, kind="ExternalOutput")

    from contextlib import ExitStack
    with tile.TileContext(nc) as tc, ExitStack() as ctx:
        _body(nc, tc, ctx, xT, wqkvT, woT, qrow, btbl, part)
    nc.finalize()
    return nc


class _K:
    """Kernel emission state: tile pools, resident tiles, and the deferral
    queues. The PE runs strictly in program order, so emission order is the
    schedule: attention tiles (whose g comes back through the ACT/Pool
    exp/mask chain) are interleaved beat-by-beat with "dense" PE work
    (projection chains, output projection) that has no cross-engine latency.
    AV matmuls pop SKEW tiles after their score."""

    def pop_av(self):
        slot, rl, i, ot, g, q = self.avq.popleft()
        _av_half(self, q, slot, rl, i, ot, g)
        if i == len(rl) - 1:
            _norm(self, q, slot, ot)

    def pop_dense(self):
        """Run one dense unit. An outproj unit of phase q reads otn columns
        written by phase q's norms, so every pending AV of phase <= q must be
        emitted first (emission order IS dependency order for the tile
        framework: a read emitted before its writer reads stale data)."""
        kind, qu, run = self.dense[0]
        if kind == "op" and self.avq and self.avq[0][5] <= qu:
            self.pop_av()
            return
        self.dense.popleft()
        run()

    def drain_av(self):
        while self.avq:
            self.pop_av()

    def drain_dense(self):
        while self.dense:
            self.pop_dense()


def _body(nc, tc, ctx, xT, wqkvT, woT, qrow, btbl, part):
    k = _K()
    k.nc = nc
    k.part = part
    k.xT = xT
    k.avq = deque()
    k.dense = deque()

    const = ctx.enter_context(tc.tile_pool(name="const", bufs=1))
    k.xtp = ctx.enter_context(tc.tile_pool(name="xt", bufs=2))
    k.stg = ctx.enter_context(tc.tile_pool(name="stg", bufs=3))
    k.gp = ctx.enter_context(tc.tile_pool(name="g", bufs=8))
    k.rcp = ctx.enter_context(tc.tile_pool(name="rc", bufs=3))
    k.bcp = ctx.enter_context(tc.tile_pool(name="bc", bufs=3))
    k.osp = ctx.enter_context(tc.tile_pool(name="ostage", bufs=4))

    # ---------- resident constants ----------------------------------------
    k.wqkv_res = const.tile([128, NE, ES + 2 * D], BF16)
    k.wo_res = const.tile([128, 2, E], BF16)
    k.qTa = []
    for j in range(HL):
        qa = const.tile([65, T], BF16, tag=f"qTa{j}")
        k.qTa.append(qa)
    k.kTa = const.tile([65, T], BF16)
    k.v_aug = const.tile([128, NT128, D + 1], BF16)
    k.btbl_t = const.tile([128, HL * NT128], F32)
    k.ident = const.tile([128, 128], BF16)
    k.otn = []
    for p2 in range(2):
        o = const.tile([128, T], BF16, tag=f"otn{p2}")
        k.otn.append(o)

    # ---------- 8 PSUM banks: (acc|po) 2 + st 4 + ot 2 --------------------
    k.pup = ctx.enter_context(tc.tile_pool(name="ps_acc", bufs=2, space="PSUM"))
    k.stp = ctx.enter_context(tc.tile_pool(name="st_ps", bufs=4, space="PSUM"))
    k.otp = ctx.enter_context(tc.tile_pool(name="ot_ps", bufs=2, space="PSUM"))

    def wdma(pl):  # weight loads, interleaved chunk-by-chunk with x at q0
        if pl == 0:
            nc.sync.dma_start(
                out=k.wqkv_res[:, 0:1, :],
                in_=bass.AP(tensor=wqkvT, offset=0,
                            ap=[[384, 128], [1, 384]]))
        elif pl in (1, 2, 5, 6):
            a, b = {1: (1, 4), 2: (4, 8), 5: (8, 12), 6: (12, 16)}[pl]
            nc.scalar.dma_start(
                out=k.wqkv_res[:, a:b, :],
                in_=bass.AP(tensor=wqkvT, offset=a * 128 * 384,
                            ap=[[384, 128], [128 * 384, b - a], [1, 384]]))
        elif pl == 3:
            # off the SP queue: small constants via the scalar engine
            for j in range(HL):
                nc.scalar.dma_start(out=k.qTa[j][64:65, :],
                                    in_=qrow[j:j + 1, :])
            nc.scalar.dma_start(out=k.btbl_t, in_=btbl[:, :])
            nc.gpsimd.memset(k.kTa[64:65, :], 1.0)
            nc.gpsimd.memset(k.v_aug[:, :, D:D + 1], 1.0)
            make_identity(nc, k.ident)
        elif pl == 4:  # Wo: first needed by outproj(0) units mid-phase 1
            nc.sync.dma_start(
                out=k.wo_res,
                in_=bass.AP(tensor=woT, offset=0,
                            ap=[[E, 128], [128 * E, 2], [1, E]]))
    k.wdma = wdma

    # bootstrap: phase 0 kv + group 0 emitted straight (DMA-paced); group 1
    # becomes dense filler so attention on slot pair (0,1) starts early
    _xt_dma(k, 0)
    steps0 = _proj_steps(k, 0)
    for step in steps0[:12]:       # kv chain+copy, transposes, g0 chain+copy
        step()
    k.dense.extend(("proj", 0, s) for s in steps0[12:])
    for q in range(NTQ):
        if q < NTQ - 1:
            _xt_dma(k, q + 1)
            ps = [("proj", q + 1, s) for s in _proj_steps(k, q + 1)]
            ops = list(k.dense)            # outproj units of q-1
            k.dense.clear()
            while ps or ops:               # round-robin merge
                if ops:
                    k.dense.append(ops.pop(0))
                if ps:
                    k.dense.append(ps.pop(0))
        _attn(k, q)
        k.drain_dense()            # any proj steps attn didn't absorb
        if q < NTQ - 1:
            _push_outproj(k, q, all_dve=(q == NTQ - 2))
    _tail_outproj(k)


def _xt_dma(k, q):
    """x column-slice load for phase q; first-e chunks split for fast start."""
    nc = k.nc
    cs = q * TQ
    xt = k.xtp.tile([128, NE, TQ], BF16, tag="xt", name=f"xt{q}")
    k.xt_cur = xt
    if q == 0:
        k.wdma(0)
        nc.gpsimd.dma_start(
            out=xt[:, 0:1, :],
            in_=bass.AP(tensor=k.xT, offset=cs, ap=[[T, 128], [1, TQ]]))
        for pl, (a, b) in ((1, (1, 4)), (2, (4, 8)), (5, (8, 12)),
                           (6, (12, 16))):
            k.wdma(pl)
            nc.sync.dma_start(
                out=xt[:, a:b, :],
                in_=bass.AP(tensor=k.xT, offset=a * 128 * T + cs,
                            ap=[[T, 128], [128 * T, b - a], [1, TQ]]))
        k.wdma(3)
    else:
        nsp = 4 if q == 1 else 2
        for pl in range(nsp):
            w = NE // nsp
            nc.sync.dma_start(
                out=xt[:, w * pl:w * pl + w, :],
                in_=bass.AP(tensor=k.xT, offset=(w * pl * 128) * T + cs,
                            ap=[[T, 128], [128 * T, w], [1, TQ]]))
            if q == 1 and pl == 0:
                k.wdma(4)


def _proj_steps(k, q):
    """Projection for phase q as a list of dense-work closures: matmul
    bundles of 4 e-chunks, copy steps, and the v transposes."""
    nc = k.nc
    cs = q * TQ
    xt = k.xt_cur
    steps = []
    state = {}

    def chain_step(grp, e0):        # 4 accumulation matmuls
        def run():
            if e0 == 0:
                c0, c1 = grp * 128, (grp + 1) * 128
                if grp == 2:
                    c0, c1 = ES, ES + 2 * D
                state[grp] = (k.pup.tile([128, TQ], F32, tag="ps",
                                         name=f"acc{q}_{grp}"), c0, c1)
            acc, c0, c1 = state[grp]
            for e in range(e0, e0 + 4):
                nc.tensor.matmul(acc, k.wqkv_res[:, e, c0:c1], xt[:, e, :],
                                 start=(e == 0), stop=(e == NE - 1))
        return run

    def qcopy(grp):                 # split PSUM drain: ACT low / DVE shifted
        def run():
            acc = state[grp][0]
            nc.scalar.copy(out=k.qTa[2 * grp][0:64, cs:cs + TQ],
                           in_=acc[0:64, :])
            nc.vector.tensor_copy(out=k.qTa[2 * grp + 1][0:64, cs:cs + TQ],
                                  in_=acc[64:128, :])
        return run

    def kvcopy():
        def run():
            acc = state[2][0]
            nc.vector.tensor_copy(out=k.kTa[0:64, cs:cs + TQ],
                                  in_=acc[0:64, :])
            stv = k.stg.tile([128, TQ], BF16, tag="stg", name=f"stv{q}")
            nc.scalar.copy(out=stv[64:128, :], in_=acc[64:128, :])
            state["stv"] = stv
        return run

    def vtrans(mm):                 # v transpose via PE: [64,128] -> [128,64]
        def run():
            stv = state["stv"]
            tr = k.stp.tile([128, TQ], BF16, tag="st", name=f"tr{q}_{mm}")
            nc.tensor.transpose(tr[:, 0:D],
                                stv[64:128, mm * 128:(mm + 1) * 128],
                                k.ident[64:128, 64:128])
            nc.vector.tensor_copy(out=k.v_aug[:, 4 * q + mm, 0:D],
                                  in_=tr[:, 0:D])
        return run

    # kv first: phase 0 runs [kv, transposes, grp0] inline so attention on
    # slot pair (0,1) can start while grp1 is still DMA-paced.
    for grp in (2, 0, 1):
        for e0 in range(0, NE, 4):
            steps.append(chain_step(grp, e0))
        steps.append(qcopy(grp) if grp < 2 else kvcopy())
        if grp == 2:
            for mm in range(0, 4, 2):
                steps.append(lambda mm=mm: (vtrans(mm)(), vtrans(mm + 1)()))
    return steps


def _score_half(k, q, slot, rl, ti):
    """Score matmul + exp + causal mask for one tile; returns the g tile."""
    nc = k.nc
    cs = q * TQ
    tk, lo, hi = rl[ti]
    st = k.stp.tile([128, TQ], F32, tag="st")
    nc.tensor.matmul(st[:, lo - cs:hi - cs],
                     k.kTa[:, tk * 128:(tk + 1) * 128],
                     k.qTa[slot][:, lo:hi], start=True, stop=True)
    g = k.gp.tile([128, TQ], BF16, tag="g")
    if ti == 0 and hi - lo < TQ:
        nc.gpsimd.memset(g, 0.0)
    nc.scalar.activation(
        out=g[:, lo - cs:hi - cs], in_=st[:, lo - cs:hi - cs], func=EXP,
        bias=k.btbl_t[:, slot * NT128 + tk:slot * NT128 + tk + 1], scale=1.0)
    if tk >= 4 * q:  # diagonal block: causal mask, keep j <= i
        d0 = tk * 128 - cs
        nc.gpsimd.affine_select(
            out=g[:, d0:d0 + 128], in_=g[:, d0:d0 + 128],
            compare_op=mybir.AluOpType.is_ge,
            fill=0.0, base=0, pattern=[[1, 128]], channel_multiplier=-1)
    return g


def _av_half(k, q, slot, rl, ti, ot, g):
    """Accumulate one tile's g @ v into the head-output PSUM."""
    nc = k.nc
    cs = q * TQ
    tk, lo, hi = rl[ti]
    a_lo, a_hi = (0, TQ) if ti == 0 else (lo - cs, hi - cs)
    nc.tensor.matmul(ot[:, a_lo:a_hi], k.v_aug[:, tk, :], g[:, a_lo:a_hi],
                     start=(ti == 0), stop=(ti == len(rl) - 1))


def _norm(k, q, slot, ot):
    """headout = ot[0:64] / ot[64]; write into otn pair layout. The
    denominator row sits at PSUM partition 64; DVE handles the shifted
    reciprocal to partition 0 and the shifted odd-half multiply directly
    (verified on HW), so no staging DMAs are needed."""
    nc = k.nc
    cs, ce = q * TQ, (q + 1) * TQ
    rc = k.rcp.tile([1, TQ], F32, tag="rc")
    nc.vector.reciprocal(out=rc[0:1, :], in_=ot[64:65, :])
    bc = k.bcp.tile([64, TQ], F32, tag="bc")
    nc.gpsimd.partition_broadcast(bc, rc[0:1, :])
    pair, half = slot // 2, slot % 2
    nc.vector.tensor_mul(out=k.otn[pair][half * 64:half * 64 + 64, cs:ce],
                         in0=ot[0:64, :], in1=bc)


def _attn(k, q):
    """Attention for all 4 slots, pairwise interleaved, beat-scheduled:
    each beat emits one score tile, pops due AVs (SKEW behind), and pops
    dense work at a rate that exhausts the dense queue with the tiles."""
    seq = []
    for sA, sB in ((0, 1), (2, 3)):
        rlA, rlB = _tk_ranges(k_q := q, WS[sA]), _tk_ranges(q, WS[sB])
        otA = k.otp.tile([65, TQ], F32, tag="ot", name=f"ot{q}_{sA}")
        otB = k.otp.tile([65, TQ], F32, tag="ot", name=f"ot{q}_{sB}")
        for i in range(max(len(rlA), len(rlB))):
            if i < len(rlA):
                seq.append((sA, rlA, i, otA))
            if i < len(rlB):
                seq.append((sB, rlB, i, otB))
    for n, (slot, rl, i, ot) in enumerate(seq):
        g = _score_half(k, q, slot, rl, i)
        k.avq.append((slot, rl, i, ot, g, q))
        if len(k.avq) > SKEW:
            k.pop_av()
        left = len(seq) - n - 1
        ndense = len(k.dense) if left == 0 else (len(k.dense) + left - 1) // left
        for _ in range(min(ndense, 3 if left else len(k.dense))):
            if k.dense:
                k.pop_dense()


def _tail_outproj(k):
    """Last phase's output projection. otn[0] (slot pair 0,1) is final
    before the last AV drain, so those half-matmuls preheat PSUM banks while
    the drain's norm chains run; otn[1] halves, copies, and split DMAs
    follow. Keeps the PE fed through the very end."""
    nc = k.nc
    t0 = 4 * (NTQ - 1)
    obs = {}

    def ensure_ob(t):
        if t not in obs:
            obs[t] = k.osp.tile([128, 4, TQ], BF16, tag="ob", name=f"tob{t}")
        return obs[t]

    def a_half(t, o):
        pool, tag = (k.pup, "ps") if (t + o) % 2 == 0 else (k.stp, "st")
        po = pool.tile([128, TQ], F32, tag=tag, name=f"tpo{t}_{o}")
        nc.tensor.matmul(po, k.otn[0][:, t * 128:(t + 1) * 128],
                         k.wo_res[:, 0, o * TQ:(o + 1) * TQ],
                         start=True, stop=False)
        return po

    def finish(t, o, po):
        nc.tensor.matmul(po, k.otn[1][:, t * 128:(t + 1) * 128],
                         k.wo_res[:, 1, o * TQ:(o + 1) * TQ],
                         start=False, stop=True)
        ob = ensure_ob(t)
        if o % 2:
            nc.scalar.copy(out=ob[:, o, :], in_=po)
        else:
            nc.vector.tensor_copy(out=ob[:, o, :], in_=po)
        if o == 1:
            nc.sync.dma_start(out=k.part[t * 128:(t + 1) * 128, 0:2 * TQ],
                              in_=ob[:, 0:2, :])
        elif o == 3:
            nc.sync.dma_start(out=k.part[t * 128:(t + 1) * 128, 2 * TQ:4 * TQ],
                              in_=ob[:, 2:4, :])

    pre = [(t0, 0), (t0, 1), (t0 + 1, 0), (t0 + 1, 1), (t0 + 2, 0), (t0 + 2, 1)]
    pos = {}
    for t, o in pre:
        pos[(t, o)] = a_half(t, o)
        for _ in range(2):
            if k.avq:
                k.pop_av()
    k.drain_av()
    for t, o in pre:
        finish(t, o, pos[(t, o)])
    rest = [(t0 + 3, 0), (t0 + 3, 1), (t0, 2), (t0, 3), (t0 + 1, 2),
            (t0 + 1, 3), (t0 + 2, 2), (t0 + 2, 3), (t0 + 3, 2), (t0 + 3, 3)]
    for t, o in rest:
        finish(t, o, a_half(t, o))


def _push_outproj(k, q, all_dve=False):
    """Queue output projection for phase q's 4 token blocks as dense units.
    These pop as filler during phase q+1; for q = 2 (popping during the
    ACT-saturated phase 3) all copies go to DVE to keep exps flowing."""
    nc = k.nc
    state = {}

    def unit(t, o, use_st):
        def run():
            if o == 0:
                state[t] = k.osp.tile([128, 4, TQ], BF16, tag="ob",
                                      name=f"ob{t}")
            ob = state[t]
            pool, tag = (k.stp, "st") if use_st else (k.pup, "ps")
            po = pool.tile([128, TQ], F32, tag=tag, name=f"po{t}_{o}")
            nc.tensor.matmul(po, k.otn[0][:, t * 128:(t + 1) * 128],
                             k.wo_res[:, 0, o * TQ:(o + 1) * TQ],
                             start=True, stop=False)
            nc.tensor.matmul(po, k.otn[1][:, t * 128:(t + 1) * 128],
                             k.wo_res[:, 1, o * TQ:(o + 1) * TQ],
                             start=False, stop=True)
            if o % 2 and not all_dve:
                nc.scalar.copy(out=ob[:, o, :], in_=po)
            else:
                nc.vector.tensor_copy(out=ob[:, o, :], in_=po)
            if o == 3:
                nc.sync.dma_start(out=k.part[t * 128:(t + 1) * 128, :],
                                  in_=ob[:, :, :])
        return run

    for t in range(4 * q, 4 * q + 4):
        for o in range(4):
            k.dense.append(("op", q, unit(t, o, False)))


def _prepare_in_maps(x, Wq, Wk, Wv, Wo):
    xTn = np.ascontiguousarray(x[0].T).astype(NPBF16)
    scale = np.float64(D) ** -0.5
    i = np.arange(T, dtype=np.float64)
    p = np.arange(128, dtype=np.float64)
    kk = np.arange(NT128, dtype=np.float64)
    in_maps = []
    for c in range(NCORES):
        hs = [24 + c, c, 16 + c, 8 + c]   # window profile WS = [16, 1, 10, 3]
        wq_rows = np.concatenate(
            [Wq[h * D:(h + 1) * D, :] * scale for h in hs], axis=0)  # [256, E]
        wkv = np.concatenate([Wk, Wv], axis=0)                       # [128, E]
        wqkvT = np.ascontiguousarray(
            np.concatenate([wq_rows, wkv], axis=0).T).astype(NPBF16)
        woT = np.ascontiguousarray(
            np.concatenate([Wo[:, h * D:(h + 1) * D] for h in hs], axis=1).T
        ).astype(NPBF16)
        slopes = np.power(2.0, -8.0 * (np.asarray(hs, np.float64) + 1.0) / H)
        qrow_n = (-slopes[:, None] * i[None, :]).astype(NPBF16)      # [HL, T]
        btbl_n = (slopes[:, None, None] * (kk[None, :, None] * 128 + p[None, None, :]))
        btbl_n = np.ascontiguousarray(
            btbl_n.transpose(2, 0, 1).reshape(128, HL * NT128)).astype(np.float32)
        in_maps.append({
            "xT": xTn, "wqkvT": wqkvT, "woT": woT,
            "qrow": qrow_n, "btbl": btbl_n,
        })
    return in_maps


def kernel(x, Wq, Wk, Wv, Wo, attention_mask, _trace=False, _trace_cores=None):
    x = np.asarray(x, dtype=np.float32)
    Wq = np.asarray(Wq, dtype=np.float32)
    Wk = np.asarray(Wk, dtype=np.float32)
    Wv = np.asarray(Wv, dtype=np.float32)
    Wo = np.asarray(Wo, dtype=np.float32)

    if "nc" not in _CACHE:
        _CACHE["nc"] = _build_nc()
    nc = _CACHE["nc"]

    in_maps = _prepare_in_maps(x, Wq, Wk, Wv, Wo)
    kwargs = {}
    if _trace:
        kwargs = {"trace": True, "trace_cores": _trace_cores or [0]}
    res = run_bass_kernel_spmd(nc, in_maps, core_ids=list(range(NCORES)), **kwargs)
    acc = np.zeros((T, E), dtype=np.float64)
    for r in res.results:
        acc += np.asarray(r["part"]).astype(np.float64)
    out = acc.astype(np.float32)[None, :, :]
    if _trace:
        _CACHE["last_result"] = res
    return out
